# revision 9
# baseline (speedup 1.0000x reference)
"""GRU decoder (teacher forcing) + log_softmax on 8 Trainium2 NeuronCores.

Strategy (v2):
  - Vocab-shard projection/log-softmax across 8 cores; replicate the serial
    GRU recurrence on every core.
  - Phase 0 (per 8-step chunk): indirect-DMA gather of embedding rows,
    DMA-xbar transpose to k-major, matmul -> x-side gate pre-acts
    XG = 16 * emb @ W_ih.T stored time-major in SBUF (the 16x matches the
    fp8 W_hh scaling so gate pre-acts share one scale).
  - Phase 1 (63 sequential steps): W_hh kept in fp8e4m3 (x16) -> FWL loads
    weight slabs at 2x bf16 rate; per gate the x-side pre-acts are folded
    into PSUM with ONE 128-col identity matmul (start=True) and the W_hh
    matmuls accumulate on top.  Burst order r -> n -> z puts the binding
    n-path dependencies earliest.  sigma(x)=0.5*tanh(x/2)+0.5 so only the
    exp_and_others ACT table is used.  h' lands in bf16 HT and (x4) in fp8
    HT8 for phase 2.
  - Phase 2 (16 row-tiles): DoubleRow fp8 matmuls (HT8 x W_projT8, both
    pre-scaled; logits in PSUM are 256x) in [128,1000] vocab units;
    VectorE evacuates raw logits (/256) to f16; ScalarE exp(l - 4ln2) with
    accum_out collects row sums.  Per 2 row-tiles one tiny AllReduce sums
    the softmax denominators across cores; -lse via DVE frexp + deg-3
    poly; out = logit + (-lse) on DVE in f16, DMA'd out as f16.
  - Collective consumption is deferred ~4 steps behind its start so the
    ~5us CC latency never blocks any engine queue.
  - Startup DMAs are spread across engine queues (sync/vector/scalar) so
    the first GRU step starts ~7us in.

kernel(**inputs) takes FULL numpy inputs, preps layouts on host, runs the
SPMD NEFF on cores 0..7 and reassembles the [32, 64, 32000] f32 output.
"""

import os

import numpy as np
import ml_dtypes

import concourse.bass as bass
import concourse.bacc as bacc
import concourse.mybir as mybir
import concourse.tile as tile
from concourse.bass_utils import run_bass_kernel_spmd
from concourse.masks import make_identity

# problem shape (hardcoded per contract)
B, T, V, E, H = 32, 64, 32000, 256, 512
S = T - 1                 # 63 decode steps
NCORES = 8
VS = V // NCORES          # 4000 vocab shard per core
G = 3 * H                 # 1536 gate dims
GC = G // 128             # 12 gate chunks
KH = H // 128             # 4 contraction tiles over H
KE = E // 128             # 2 contraction tiles over E
NROW = S * B              # 2016 output rows, (t, b) order
NMT = (NROW + 127) // 128  # 16 row-tiles (last has 96 rows)
NGRP = 8                  # stat-collective groups (2 row-tiles each)
MPG = NMT // NGRP         # 2 row-tiles per group
VU = 1000                 # vocab unit for psum/exp
NVU = VS // VU            # 4 units per row-tile
LN2 = float(np.log(2.0))
EXP_BIAS = -4.0 * LN2     # exp(logit - 4ln2): keeps fp16 exp safely in range
WHH_SCALE = 16.0          # fp8 W_hh (and x-gate) pre-scale
HT8_SCALE = 4.0           # fp8 h pre-scale for phase 2
WPR_SCALE = 64.0          # fp8 W_proj pre-scale
LG_SCALE = HT8_SCALE * WPR_SCALE   # phase-2 PSUM logits are 256x

F32 = mybir.dt.float32
BF16 = mybir.dt.bfloat16
F16 = mybir.dt.float16
FP8 = mybir.dt.float8e4
I32 = mybir.dt.int32
U32 = mybir.dt.uint32
AF = mybir.ActivationFunctionType
OP = mybir.AluOpType
DR = mybir.MatmulPerfMode.DoubleRow
NP_FP8 = ml_dtypes.float8_e4m3fn

# -ln(m) Chebyshev-interpolation coefficients on m in [1, 2], highest first.
_nodes = np.cos((2 * np.arange(1, 5) - 1) / (2 * 4.0) * np.pi) * 0.5 + 1.5
_NEGLN_COEF = [float(c) for c in np.polyfit(_nodes, -np.log(_nodes), 3)]

_BUILD_CACHE = {}


def _build(bx_nonzero: bool, bhh_n_nonzero: bool, bproj_nonzero: bool):
    debug = bool(int(os.environ.get("KERNEL_DEBUG", "0")))
    use_dr = bool(int(os.environ.get("KERNEL_DR", "1")))
    ht8_gps = bool(int(os.environ.get("KERNEL_HT8GPS", "0")))
    key = (bx_nonzero, bhh_n_nonzero, bproj_nonzero, debug, use_dr, ht8_gps)
    if key in _BUILD_CACHE:
        return _BUILD_CACHE[key]

    nc = bacc.Bacc("TRN2", target_bir_lowering=False, debug=False,
                   enable_asserts=False, num_devices=NCORES)

    trg_d = nc.dram_tensor("trg_flat", (NROW, 1), I32, kind="ExternalInput")
    tbl_d = nc.dram_tensor("emb_tbl", (V, E), BF16, kind="ExternalInput")
    wih_d = nc.dram_tensor("wih_t", (128, KE, G), BF16, kind="ExternalInput")
    whh_d = nc.dram_tensor("whh_t", (128, KH, G), FP8, kind="ExternalInput")
    h0_d = nc.dram_tensor("h0_t", (128, KH, B), BF16, kind="ExternalInput")
    wpr_d = nc.dram_tensor("wproj_t", (128, KH, VS), FP8, kind="ExternalInput")
    if bx_nonzero:
        bx_d = nc.dram_tensor("bx_t", (128, GC), BF16, kind="ExternalInput")
    if bhh_n_nonzero:
        bhn_d = nc.dram_tensor("bhn_t", (128, KH), BF16, kind="ExternalInput")
    if bproj_nonzero:
        bpr_d = nc.dram_tensor("bproj_s", (1, VS), F32, kind="ExternalInput")
    out_d = nc.dram_tensor("out_lp", (NROW, VS), F16, kind="ExternalOutput")
    if debug:
        ht_d = nc.dram_tensor("dbg_ht", (128, KH, NROW), BF16,
                              kind="ExternalOutput")
        sall_d = nc.dram_tensor("dbg_sall", (128, NMT * NVU), F32,
                                kind="ExternalOutput")
        lg_d = nc.dram_tensor("dbg_lg", (128, VS), F16, kind="ExternalOutput")
        nlse_d = nc.dram_tensor("dbg_nlse", (128, MPG), F32,
                                kind="ExternalOutput")

    with tile.TileContext(nc) as tc:
        with tc.tile_pool(name="sb", bufs=1) as sb, \
             tc.tile_pool(name="ps", bufs=1, space="PSUM") as ps, \
             tc.tile_pool(name="dram", bufs=1, space="DRAM") as dp:

            # ---------- phase-0 prep for chunk 0 first (critical path) ------
            ebias = sb.tile([128, 1], F32)
            nc.gpsimd.memset(ebias[:], EXP_BIAS)
            ident = sb.tile([128, 128], BF16)
            make_identity(nc, ident[:])
            S_all = sb.tile([128, NMT * NVU], F32)   # exp partial sums
            nc.gpsimd.memset(S_all[:], 0.0)

            xg_tiles = {}

            def emit_prep_gather(c8):
                tlo = 8 * c8
                nst = min(8, S - tlo)
                nrows = B * nst
                xg = sb.tile([128, 8, GC, B], BF16, tag="xg", bufs=2,
                             name=f"xg{c8}")
                xg_tiles[c8] = xg
                embt = sb.tile([128, KE, 256], BF16, tag="embt", bufs=2,
                               name=f"embt{c8}")
                for sub in range(2):
                    lo = tlo * B + sub * 128
                    nr = min(128, nrows - sub * 128)
                    if nr <= 0:
                        continue
                    idx_t = sb.tile([128, 1], I32, tag="idx", bufs=4,
                                    name=f"idx{c8}_{sub}")
                    nc.sync.dma_start(idx_t[:nr], trg_d[lo:lo + nr, :])
                    rows = sb.tile([128, E], BF16, tag="embr", bufs=4,
                                   name=f"embr{c8}_{sub}")
                    nc.gpsimd.indirect_dma_start(
                        out=rows[:nr], out_offset=None, in_=tbl_d[:],
                        in_offset=bass.IndirectOffsetOnAxis(ap=idx_t[:nr, :1], axis=0))
                    for kb in range(KE):
                        nc.sync.dma_start_transpose(
                            embt[:, kb, sub * 128:sub * 128 + nr],
                            rows[:nr, kb * 128:(kb + 1) * 128])
                return embt

            def emit_prep_xg(c8, embt, gcs):
                tlo = 8 * c8
                nst = min(8, S - tlo)
                nrows = B * nst
                xg = xg_tiles[c8]
                for gc in gcs:
                    pxg = ps.tile([128, 256], F32, tag="ps_xg", bufs=1,
                                  name=f"pxg{c8}_{gc}")
                    for kt in range(KE):
                        nc.tensor.matmul(
                            pxg[:, :nrows],
                            lhsT=wih_sb[:, kt, gc * 128:(gc + 1) * 128],
                            rhs=embt[:, kt, :nrows],
                            start=(kt == 0), stop=(kt == KE - 1))
                    src = pxg[:, :nrows].rearrange("p (t b) -> p t b", b=B)
                    if bx_nonzero:
                        nc.vector.tensor_tensor(
                            out=xg[:, :nst, gc, :], in0=src,
                            in1=bx_sb[:, gc:gc + 1].to_broadcast([128, nst, B]),
                            op=OP.add)
                    else:
                        nc.vector.tensor_copy(xg[:, :nst, gc, :], src)

            embt0 = emit_prep_gather(0)

            # ---------- persistent loads spread over engine DMA queues ------
            wih_sb = sb.tile([128, KE, G], BF16)
            nc.scalar.dma_start(wih_sb[:], wih_d[:])
            whh_sb = sb.tile([128, KH, G], FP8)
            nc.scalar.dma_start(whh_sb[:], whh_d[:])
            h0_sb = sb.tile([128, KH, B], BF16)
            nc.scalar.dma_start(h0_sb[:], h0_d[:])
            wpr_sb = sb.tile([128, KH, VS], FP8)
            nc.scalar.dma_start(wpr_sb[:], wpr_d[:])
            if bx_nonzero:
                bx_sb = sb.tile([128, GC], BF16)
                nc.gpsimd.dma_start(bx_sb[:], bx_d[:])
            if bhh_n_nonzero:
                bhn_sb = sb.tile([128, KH], BF16)
                nc.gpsimd.dma_start(bhn_sb[:], bhn_d[:])
            if bproj_nonzero:
                bpr_sb = sb.tile([128, VS], F32)
                nc.gpsimd.dma_start(bpr_sb[:], bpr_d[:1, :].to_broadcast([128, VS]))

            HT = sb.tile([128, KH, NROW], BF16)     # h_{t+1} states, (t, b) cols
            HT8 = sb.tile([128, KH, NROW], FP8)     # 4x h in fp8 for phase 2

            # warm up the collective path (first CC op pays ~25us extra)
            warm_in = dp.tile([128, 1], F32, tag="warm_in")
            warm_out = dp.tile([128, 1], F32, tag="warm_out",
                               addr_space="Shared")
            nc.gpsimd.dma_start(warm_in[:], ebias[:])
            nc.gpsimd.collective_compute(
                "AllReduce", OP.add, replica_groups=[list(range(NCORES))],
                ins=[warm_in.opt()], outs=[warm_out.opt()])

            emit_prep_xg(0, embt0, range(GC))

            # ---------------- phase 1 step ----------------------------------
            r_off, z_off, n_off = 0, 4, 8   # gate chunk offsets (r, z, n)

            def emit_step(t):
                h_prev = h0_sb[:, :, :] if t == 0 else HT[:, :, (t - 1) * B:t * B]
                xg = xg_tiles[t // 8][:, t % 8, :, :]
                ps_r = ps.tile([128, 4, B], F32, tag="ps_r", name=f"psr{t}")
                ps_n = ps.tile([128, 4, B], F32, tag="ps_n", name=f"psn{t}")
                ps_z = ps.tile([128, 4, B], F32, tag="ps_z", name=f"psz{t}")

                def gate_mms(dst, off, fold):
                    if fold:
                        nc.tensor.matmul(dst[:], lhsT=ident[:],
                                         rhs=xg[:, off:off + 4, :],
                                         start=True, stop=False)
                    for gc in range(4):
                        for kt in range(KH):
                            nc.tensor.matmul(
                                dst[:, gc, :],
                                lhsT=whh_sb[:, kt, (off + gc) * 128:(off + gc + 1) * 128],
                                rhs=h_prev[:, kt, :],
                                start=(not fold and kt == 0),
                                stop=(kt == KH - 1))

                # burst order: r first (binding n-path needs r_s), then n,
                # then z (its consumers have the most slack).
                gate_mms(ps_r, r_off, True)
                gate_mms(ps_n, n_off, False)
                gate_mms(ps_z, z_off, True)

                # r gate: sigma(x) = 0.5*tanh(x/2) + 0.5  (pre-acts are 16x)
                rt = sb.tile([128, 4, B], BF16, tag="rt", bufs=2, name=f"rt{t}")
                nc.scalar.activation(rt[:], ps_r[:], AF.Tanh,
                                     scale=0.5 / WHH_SCALE)
                r_s = sb.tile([128, 4, B], BF16, tag="r_s", bufs=2, name=f"rs{t}")
                nc.vector.tensor_scalar(out=r_s[:], in0=rt[:], scalar1=0.5,
                                        scalar2=0.5, op0=OP.mult, op1=OP.add)
                # z gate
                zt = sb.tile([128, 4, B], BF16, tag="zt", bufs=2, name=f"zt{t}")
                nc.scalar.activation(zt[:], ps_z[:], AF.Tanh,
                                     scale=0.5 / WHH_SCALE)
                q_s = sb.tile([128, 4, B], BF16, tag="q_s", bufs=2, name=f"qs{t}")
                nc.gpsimd.tensor_scalar(out=q_s[:], in0=zt[:], scalar1=-0.5,
                                        scalar2=0.5, op0=OP.mult, op1=OP.add)
                z_s = sb.tile([128, 4, B], BF16, tag="z_s", bufs=2, name=f"zs{t}")
                nc.gpsimd.tensor_scalar(out=z_s[:], in0=zt[:], scalar1=0.5,
                                        scalar2=0.5, op0=OP.mult, op1=OP.add)
                p_s = sb.tile([128, 4, B], BF16, tag="p_s", bufs=2, name=f"ps{t}")
                nc.gpsimd.tensor_tensor(out=p_s[:], in0=z_s[:], in1=h_prev,
                                        op=OP.mult)
                # n gate: tanh((16*xn + r*(16*hn)) / 16)
                if bhh_n_nonzero:
                    nc.vector.tensor_tensor(
                        out=ps_n[:], in0=ps_n[:],
                        in1=bhn_sb[:, :, None].to_broadcast([128, 4, B]), op=OP.add)
                nc.vector.tensor_tensor(out=ps_n[:], in0=ps_n[:], in1=r_s[:],
                                        op=OP.mult)
                nc.vector.tensor_tensor(out=ps_n[:], in0=ps_n[:],
                                        in1=xg[:, n_off:n_off + 4, :], op=OP.add)
                n_s = sb.tile([128, 4, B], BF16, tag="n_s", bufs=2, name=f"ns{t}")
                nc.scalar.activation(n_s[:], ps_n[:], AF.Tanh,
                                     scale=1.0 / WHH_SCALE)
                # h' = n*(1-z) + z*h
                w_s = sb.tile([128, 4, B], BF16, tag="w_s", bufs=2, name=f"ws{t}")
                nc.vector.tensor_tensor(out=w_s[:], in0=n_s[:], in1=q_s[:],
                                        op=OP.mult)
                nc.vector.tensor_tensor(out=HT[:, :, t * B:(t + 1) * B],
                                        in0=w_s[:], in1=p_s[:], op=OP.add)
                # fp8 copy (x4) for phase 2, off the binding path
                eng = nc.gpsimd if ht8_gps else nc.vector
                eng.tensor_scalar(out=HT8[:, :, t * B:(t + 1) * B],
                                  in0=HT[:, :, t * B:(t + 1) * B],
                                  scalar1=HT8_SCALE, scalar2=None,
                                  op0=OP.mult)

            # ---------------- phase 2 emission helpers ----------------------
            logit_tiles = {}
            lse_tiles = {}

            def emit_munit(m, u):
                # one 1000-vocab unit of row-tile m: DoubleRow fp8 matmuls,
                # f16 raw-logit evacuation, exp + row-sum accumulation.
                mp = min(128, NROW - m * 128)
                if u == 0:
                    logit_tiles[m] = sb.tile([128, VS], F16, tag="logit",
                                             bufs=5, name=f"lg{m}")
                lg = logit_tiles[m]
                pl = ps.tile([128, VU], F32, tag="ps_l", bufs=2,
                             name=f"pl{m}_{u}")
                if use_dr:
                    for khi in range(2):
                        for half in range(2):
                            nc.tensor.matmul(
                                pl[:mp, half * 500:(half + 1) * 500],
                                lhsT=HT8[:, 2 * khi:2 * khi + 2,
                                         m * 128:m * 128 + mp],
                                rhs=wpr_sb[:, 2 * khi:2 * khi + 2,
                                           u * VU + half * 500:u * VU + (half + 1) * 500],
                                start=(khi == 0), stop=(khi == 1),
                                perf_mode=DR)
                else:
                    for kt in range(KH):
                        nc.tensor.matmul(
                            pl[:mp],
                            lhsT=HT8[:, kt, m * 128:m * 128 + mp],
                            rhs=wpr_sb[:, kt, u * VU:(u + 1) * VU],
                            start=(kt == 0), stop=(kt == KH - 1))
                if bproj_nonzero:
                    nc.vector.tensor_tensor(
                        out=pl[:mp], in0=pl[:mp],
                        in1=bpr_sb[:mp, u * VU:(u + 1) * VU], op=OP.add)
                nc.vector.tensor_scalar(
                    out=lg[:mp, u * VU:(u + 1) * VU], in0=pl[:mp],
                    scalar1=1.0 / LG_SCALE, scalar2=None, op0=OP.mult)
                esc = sb.tile([128, VU], F16, tag="exps", bufs=2,
                              name=f"esc{m}_{u}")
                nc.scalar.activation(esc[:mp], pl[:mp], AF.Exp,
                                     bias=ebias[:mp, :1], scale=1.0 / LG_SCALE,
                                     accum_out=S_all[:mp, m * NVU + u:m * NVU + u + 1])

            cc_tiles = {}

            def emit_stats_start(g):
                # local row-sums for the group's 2 row-tiles + AllReduce
                sg = sb.tile([128, MPG], F32, tag="sg", bufs=2, name=f"sg{g}")
                for j in range(MPG):
                    m = g * MPG + j
                    nc.vector.reduce_sum(
                        out=sg[:, j:j + 1],
                        in_=S_all[:, m * NVU:(m + 1) * NVU],
                        axis=mybir.AxisListType.X)
                cin = dp.tile([128, MPG], F32, tag=f"cin{g}", name=f"cin{g}")
                nc.gpsimd.dma_start(cin[:], sg[:])
                cout = dp.tile([128, MPG], F32, tag=f"cout{g}",
                               addr_space="Shared", name=f"cout{g}")
                nc.gpsimd.collective_compute(
                    "AllReduce", OP.add,
                    replica_groups=[list(range(NCORES))],
                    ins=[cin.opt()], outs=[cout.opt()])
                cc_tiles[g] = cout

            def emit_stats_consume(g):
                st = sb.tile([128, MPG], F32, tag="st", bufs=2, name=f"st{g}")
                nc.sync.dma_start(st[:], cc_tiles[g][:])
                # neg_lse = -(e - 127 + 4) * ln2 - ln(m),  St = m * 2^(e-127)
                iu = st[:].bitcast(U32)
                eu = sb.tile([128, MPG], U32, tag="eu", bufs=2, name=f"eu{g}")
                nc.vector.tensor_scalar(out=eu[:], in0=iu, scalar1=23,
                                        scalar2=None, op0=OP.logical_shift_right)
                ef = sb.tile([128, MPG], F32, tag="ef", bufs=2, name=f"ef{g}")
                nc.vector.tensor_copy(ef[:], eu[:])
                mu = sb.tile([128, MPG], U32, tag="mu", bufs=2, name=f"mu{g}")
                nc.vector.tensor_scalar(out=mu[:], in0=iu, scalar1=0x007FFFFF,
                                        scalar2=0x3F800000, op0=OP.bitwise_and,
                                        op1=OP.bitwise_or)
                mf = mu[:].bitcast(F32)
                acc = sb.tile([128, MPG], F32, tag="acc", bufs=2, name=f"acc{g}")
                c = _NEGLN_COEF
                nc.vector.tensor_scalar(out=acc[:], in0=mf, scalar1=c[0],
                                        scalar2=c[1], op0=OP.mult, op1=OP.add)
                for k in range(2, 4):
                    nc.vector.tensor_tensor(out=acc[:], in0=acc[:], in1=mf,
                                            op=OP.mult)
                    nc.vector.tensor_scalar(out=acc[:], in0=acc[:], scalar1=c[k],
                                            scalar2=None, op0=OP.add)
                # + (127 - 4 - e) * ln2   (the -4 re-adds the exp bias)
                e2 = sb.tile([128, MPG], F32, tag="e2", bufs=2, name=f"e2{g}")
                nc.vector.tensor_scalar(out=e2[:], in0=ef[:], scalar1=-LN2,
                                        scalar2=(127.0 - 4.0) * LN2,
                                        op0=OP.mult, op1=OP.add)
                nlse = sb.tile([128, MPG], F32, tag="nlse", bufs=2,
                               name=f"nlse{g}")
                nc.vector.tensor_tensor(out=nlse[:], in0=acc[:], in1=e2[:],
                                        op=OP.add)
                lse_tiles[g] = nlse
                if debug and g == 0:
                    nc.sync.dma_start(nlse_d[:], nlse[:])

            def emit_output(m):
                g, j = m // MPG, m % MPG
                mp = min(128, NROW - m * 128)
                nlse = lse_tiles[g]
                lg = logit_tiles.pop(m)
                if debug and m == 0:
                    nc.sync.dma_start(lg_d[:], lg[:])
                ot = sb.tile([128, VS], F16, tag="ot", bufs=2, name=f"ot{m}")
                nc.vector.tensor_tensor(
                    out=ot[:mp], in0=lg[:mp],
                    in1=nlse[:mp, j:j + 1].to_broadcast([mp, VS]), op=OP.add)
                nc.sync.dma_start(out_d[m * 128:m * 128 + mp, :], ot[:mp])

            # ---------------- main emission loop ----------------------------
            from collections import deque
            work_q = deque()

            def enqueue_mtile(m):
                # deferred consume of the group started ~4 steps ago: its
                # AllReduce has been in flight since then.
                if m >= 2 and m % 2 == 0:
                    g = (m - 2) // 2

                    def fin(g=g):
                        emit_stats_consume(g)
                        for mm in range(g * MPG, (g + 1) * MPG):
                            emit_output(mm)
                    work_q.append(fin)
                for u in range(NVU):
                    work_q.append(lambda m=m, u=u: emit_munit(m, u))
                if m % 2 == 1:
                    work_q.append(lambda g=m // 2: emit_stats_start(g))

            for t in range(S):
                emit_step(t)
                if t % 8 == 1 and t // 8 + 1 <= (S - 1) // 8:
                    c8 = t // 8 + 1
                    embt = emit_prep_gather(c8)
                    for lo in range(0, GC, 3):
                        work_q.append(lambda c8=c8, embt=embt, lo=lo:
                                      emit_prep_xg(c8, embt, range(lo, min(lo + 3, GC))))
                if t >= 3 and (t - 3) % 4 == 0:
                    enqueue_mtile((t - 3) // 4)
                for _ in range(min(3, len(work_q))):
                    work_q.popleft()()
            # tail: tile 15, last group stats, remaining consumes/outputs
            enqueue_mtile(15)
            work_q.append(lambda: emit_stats_consume(7))
            work_q.append(lambda: emit_output(14))
            work_q.append(lambda: emit_output(15))
            while work_q:
                work_q.popleft()()
            if debug:
                nc.sync.dma_start(ht_d[:], HT[:])
                nc.sync.dma_start(sall_d[:], S_all[:])

    nc.finalize()
    _BUILD_CACHE[key] = nc
    return nc


def _pack_T(w, ktiles, scale, np_dt):
    """[out, in] f32 -> [128, ktiles, out] (w.T * scale, k-major slabs)."""
    wT = np.ascontiguousarray(w.T) * scale
    return np.ascontiguousarray(
        wT.reshape(ktiles, 128, w.shape[0]).transpose(1, 0, 2)).astype(np_dt)


LAST_PROFILE = None


def kernel(trg, h0, embed_table, W_ih, W_hh, b_ih, b_hh, W_proj, b_proj):
    global LAST_PROFILE
    trg = np.asarray(trg)
    h0 = np.asarray(h0, dtype=np.float32)
    embed_table = np.asarray(embed_table, dtype=np.float32)
    W_ih = np.asarray(W_ih, dtype=np.float32)
    W_hh = np.asarray(W_hh, dtype=np.float32)
    b_ih = np.asarray(b_ih, dtype=np.float32)
    b_hh = np.asarray(b_hh, dtype=np.float32)
    W_proj = np.asarray(W_proj, dtype=np.float32)
    b_proj = np.asarray(b_proj, dtype=np.float32)

    # bx = b_ih + [b_hh for r,z chunks; 0 for n chunks], scaled like xg
    bx = b_ih.copy()
    bx[:2 * H] += b_hh[:2 * H]
    bx_nonzero = bool(np.any(bx))
    bhh_n_nonzero = bool(np.any(b_hh[2 * H:]))
    bproj_nonzero = bool(np.any(b_proj))
    nc = _build(bx_nonzero, bhh_n_nonzero, bproj_nonzero)

    trg_flat = np.ascontiguousarray(
        trg[:, :S].T.reshape(NROW, 1)).astype(np.int32)
    tbl_bf = embed_table.astype(ml_dtypes.bfloat16)
    wih_t = _pack_T(W_ih, KE, WHH_SCALE, ml_dtypes.bfloat16)
    whh_t = _pack_T(W_hh, KH, WHH_SCALE, NP_FP8)
    h0_t = np.ascontiguousarray(
        h0[0].T.reshape(KH, 128, B).transpose(1, 0, 2)).astype(ml_dtypes.bfloat16)

    base = {
        "trg_flat": trg_flat,
        "emb_tbl": tbl_bf,
        "wih_t": wih_t,
        "whh_t": whh_t,
        "h0_t": h0_t,
    }
    if bx_nonzero:
        base["bx_t"] = np.ascontiguousarray(
            (bx * WHH_SCALE).reshape(GC, 128).T).astype(ml_dtypes.bfloat16)
    if bhh_n_nonzero:
        base["bhn_t"] = np.ascontiguousarray(
            (b_hh[2 * H:] * WHH_SCALE).reshape(KH, 128).T).astype(ml_dtypes.bfloat16)

    in_maps = []
    for c in range(NCORES):
        m = dict(base)
        m["wproj_t"] = _pack_T(W_proj[c * VS:(c + 1) * VS], KH, WPR_SCALE,
                               NP_FP8)
        if bproj_nonzero:
            m["bproj_s"] = np.ascontiguousarray(
                (b_proj[c * VS:(c + 1) * VS] * LG_SCALE).reshape(1, VS))
        in_maps.append(m)

    trace = bool(int(os.environ.get("KERNEL_TRACE", "0")))
    res = run_bass_kernel_spmd(nc, in_maps, core_ids=list(range(NCORES)),
                               trace=trace)
    LAST_PROFILE = res

    out = np.zeros((B, T, V), dtype=np.float32)
    big = np.stack([res.results[c]["out_lp"].astype(np.float32).reshape(S, B, VS)
                    for c in range(NCORES)], axis=0)   # [c, t, b, vs]
    out[:, 1:, :] = big.transpose(2, 1, 0, 3).reshape(B, S, V)
    return out


# revision 10
# speedup vs baseline: 1.0079x; 1.0079x over previous
"""GRU decoder (teacher forcing) + log_softmax on 8 Trainium2 NeuronCores.

Strategy (v2):
  - Vocab-shard projection/log-softmax across 8 cores; replicate the serial
    GRU recurrence on every core.
  - Phase 0 (per 8-step chunk): indirect-DMA gather of embedding rows,
    DMA-xbar transpose to k-major, matmul -> x-side gate pre-acts
    XG = 16 * emb @ W_ih.T stored time-major in SBUF (the 16x matches the
    fp8 W_hh scaling so gate pre-acts share one scale).
  - Phase 1 (63 sequential steps): W_hh kept in fp8e4m3 (x16) -> FWL loads
    weight slabs at 2x bf16 rate; per gate the x-side pre-acts are folded
    into PSUM with ONE 128-col identity matmul (start=True) and the W_hh
    matmuls accumulate on top.  Burst order r -> n -> z puts the binding
    n-path dependencies earliest.  sigma(x)=0.5*tanh(x/2)+0.5 so only the
    exp_and_others ACT table is used.  h' lands in bf16 HT and (x4) in fp8
    HT8 for phase 2.
  - Phase 2 (16 row-tiles): DoubleRow fp8 matmuls (HT8 x W_projT8, both
    pre-scaled; logits in PSUM are 256x) in [128,1000] vocab units;
    VectorE evacuates raw logits (/256) to f16; ScalarE exp(l - 4ln2) with
    accum_out collects row sums.  Per 2 row-tiles one tiny AllReduce sums
    the softmax denominators across cores; -lse via DVE frexp + deg-3
    poly; out = logit + (-lse) on DVE in f16, DMA'd out as f16.
  - Collective consumption is deferred ~4 steps behind its start so the
    ~5us CC latency never blocks any engine queue.
  - Startup DMAs are spread across engine queues (sync/vector/scalar) so
    the first GRU step starts ~7us in.

kernel(**inputs) takes FULL numpy inputs, preps layouts on host, runs the
SPMD NEFF on cores 0..7 and reassembles the [32, 64, 32000] f32 output.
"""

import os

import numpy as np
import ml_dtypes

import concourse.bass as bass
import concourse.bacc as bacc
import concourse.mybir as mybir
import concourse.tile as tile
from concourse.bass_utils import run_bass_kernel_spmd
from concourse.masks import make_identity

# problem shape (hardcoded per contract)
B, T, V, E, H = 32, 64, 32000, 256, 512
S = T - 1                 # 63 decode steps
NCORES = 8
VS = V // NCORES          # 4000 vocab shard per core
G = 3 * H                 # 1536 gate dims
GC = G // 128             # 12 gate chunks
KH = H // 128             # 4 contraction tiles over H
KE = E // 128             # 2 contraction tiles over E
NROW = S * B              # 2016 output rows, (t, b) order
NMT = (NROW + 127) // 128  # 16 row-tiles (last has 96 rows)
NGRP = 8                  # stat-collective groups (2 row-tiles each)
MPG = NMT // NGRP         # 2 row-tiles per group
VU = 1000                 # vocab unit for psum/exp
NVU = VS // VU            # 4 units per row-tile
LN2 = float(np.log(2.0))
EXP_BIAS = -4.0 * LN2     # exp(logit - 4ln2): keeps fp16 exp safely in range
WHH_SCALE = 16.0          # fp8 W_hh (and x-gate) pre-scale
HT8_SCALE = 4.0           # fp8 h pre-scale for phase 2
WPR_SCALE = 64.0          # fp8 W_proj pre-scale
LG_SCALE = HT8_SCALE * WPR_SCALE   # phase-2 PSUM logits are 256x

F32 = mybir.dt.float32
BF16 = mybir.dt.bfloat16
F16 = mybir.dt.float16
FP8 = mybir.dt.float8e4
I32 = mybir.dt.int32
U32 = mybir.dt.uint32
AF = mybir.ActivationFunctionType
OP = mybir.AluOpType
DR = mybir.MatmulPerfMode.DoubleRow
NP_FP8 = ml_dtypes.float8_e4m3fn

# -ln(m) Chebyshev-interpolation coefficients on m in [1, 2], highest first.
_nodes = np.cos((2 * np.arange(1, 5) - 1) / (2 * 4.0) * np.pi) * 0.5 + 1.5
_NEGLN_COEF = [float(c) for c in np.polyfit(_nodes, -np.log(_nodes), 3)]

_BUILD_CACHE = {}


def _build(bx_nonzero: bool, bhh_n_nonzero: bool, bproj_nonzero: bool):
    debug = bool(int(os.environ.get("KERNEL_DEBUG", "0")))
    use_dr = bool(int(os.environ.get("KERNEL_DR", "1")))
    ht8_gps = bool(int(os.environ.get("KERNEL_HT8GPS", "0")))
    key = (bx_nonzero, bhh_n_nonzero, bproj_nonzero, debug, use_dr, ht8_gps)
    if key in _BUILD_CACHE:
        return _BUILD_CACHE[key]

    nc = bacc.Bacc("TRN2", target_bir_lowering=False, debug=False,
                   enable_asserts=False, num_devices=NCORES)

    trg_d = nc.dram_tensor("trg_flat", (NROW, 1), I32, kind="ExternalInput")
    tbl_d = nc.dram_tensor("emb_tbl", (V, E), BF16, kind="ExternalInput")
    wih_d = nc.dram_tensor("wih_t", (128, KE, G), BF16, kind="ExternalInput")
    whh_d = nc.dram_tensor("whh_t", (128, KH, G), FP8, kind="ExternalInput")
    h0_d = nc.dram_tensor("h0_t", (128, KH, B), BF16, kind="ExternalInput")
    wpr_d = nc.dram_tensor("wproj_t", (128, KH, VS), FP8, kind="ExternalInput")
    if bx_nonzero:
        bx_d = nc.dram_tensor("bx_t", (128, GC), BF16, kind="ExternalInput")
    if bhh_n_nonzero:
        bhn_d = nc.dram_tensor("bhn_t", (128, KH), BF16, kind="ExternalInput")
    if bproj_nonzero:
        bpr_d = nc.dram_tensor("bproj_s", (1, VS), F32, kind="ExternalInput")
    out_d = nc.dram_tensor("out_lp", (NROW, VS), F16, kind="ExternalOutput")
    if debug:
        ht_d = nc.dram_tensor("dbg_ht", (128, KH, NROW), BF16,
                              kind="ExternalOutput")
        sall_d = nc.dram_tensor("dbg_sall", (128, NMT * NVU), F32,
                                kind="ExternalOutput")
        lg_d = nc.dram_tensor("dbg_lg", (128, VS), F16, kind="ExternalOutput")
        nlse_d = nc.dram_tensor("dbg_nlse", (128, MPG), F32,
                                kind="ExternalOutput")

    with tile.TileContext(nc) as tc:
        with tc.tile_pool(name="sb", bufs=1) as sb, \
             tc.tile_pool(name="ps", bufs=1, space="PSUM") as ps, \
             tc.tile_pool(name="dram", bufs=1, space="DRAM") as dp:

            # ---------- phase-0 prep for chunk 0 first (critical path) ------
            ebias = sb.tile([128, 1], F32)
            nc.gpsimd.memset(ebias[:], EXP_BIAS)
            ident = sb.tile([128, 128], BF16)
            make_identity(nc, ident[:])
            S_all = sb.tile([128, NMT * NVU], F32)   # exp partial sums
            nc.gpsimd.memset(S_all[:], 0.0)

            xg_tiles = {}

            def emit_prep_gather(c8):
                tlo = 8 * c8
                nst = min(8, S - tlo)
                nrows = B * nst
                xg = sb.tile([128, 8, GC, B], BF16, tag="xg", bufs=2,
                             name=f"xg{c8}")
                xg_tiles[c8] = xg
                embt = sb.tile([128, KE, 256], BF16, tag="embt", bufs=2,
                               name=f"embt{c8}")
                for sub in range(2):
                    lo = tlo * B + sub * 128
                    nr = min(128, nrows - sub * 128)
                    if nr <= 0:
                        continue
                    idx_t = sb.tile([128, 1], I32, tag="idx", bufs=4,
                                    name=f"idx{c8}_{sub}")
                    nc.sync.dma_start(idx_t[:nr], trg_d[lo:lo + nr, :])
                    rows = sb.tile([128, E], BF16, tag="embr", bufs=4,
                                   name=f"embr{c8}_{sub}")
                    nc.gpsimd.indirect_dma_start(
                        out=rows[:nr], out_offset=None, in_=tbl_d[:],
                        in_offset=bass.IndirectOffsetOnAxis(ap=idx_t[:nr, :1], axis=0))
                    for kb in range(KE):
                        nc.sync.dma_start_transpose(
                            embt[:, kb, sub * 128:sub * 128 + nr],
                            rows[:nr, kb * 128:(kb + 1) * 128])
                return embt

            def emit_prep_xg(c8, embt, gcs):
                tlo = 8 * c8
                nst = min(8, S - tlo)
                nrows = B * nst
                xg = xg_tiles[c8]
                for gc in gcs:
                    pxg = ps.tile([128, 256], F32, tag="ps_xg", bufs=1,
                                  name=f"pxg{c8}_{gc}")
                    for kt in range(KE):
                        nc.tensor.matmul(
                            pxg[:, :nrows],
                            lhsT=wih_sb[:, kt, gc * 128:(gc + 1) * 128],
                            rhs=embt[:, kt, :nrows],
                            start=(kt == 0), stop=(kt == KE - 1))
                    src = pxg[:, :nrows].rearrange("p (t b) -> p t b", b=B)
                    if bx_nonzero:
                        nc.vector.tensor_tensor(
                            out=xg[:, :nst, gc, :], in0=src,
                            in1=bx_sb[:, gc:gc + 1].to_broadcast([128, nst, B]),
                            op=OP.add)
                    else:
                        nc.vector.tensor_copy(xg[:, :nst, gc, :], src)

            embt0 = emit_prep_gather(0)

            # ---------- persistent loads spread over engine DMA queues ------
            wih_sb = sb.tile([128, KE, G], BF16)
            nc.scalar.dma_start(wih_sb[:], wih_d[:])
            whh_sb = sb.tile([128, KH, G], FP8)
            nc.scalar.dma_start(whh_sb[:], whh_d[:])
            h0_sb = sb.tile([128, KH, B], BF16)
            nc.scalar.dma_start(h0_sb[:], h0_d[:])
            wpr_sb = sb.tile([128, KH, VS], FP8)
            nc.scalar.dma_start(wpr_sb[:], wpr_d[:])
            if bx_nonzero:
                bx_sb = sb.tile([128, GC], BF16)
                nc.gpsimd.dma_start(bx_sb[:], bx_d[:])
            if bhh_n_nonzero:
                bhn_sb = sb.tile([128, KH], BF16)
                nc.gpsimd.dma_start(bhn_sb[:], bhn_d[:])
            if bproj_nonzero:
                bpr_sb = sb.tile([128, VS], F32)
                nc.gpsimd.dma_start(bpr_sb[:], bpr_d[:1, :].to_broadcast([128, VS]))

            HT = sb.tile([128, KH, NROW], BF16)     # h_{t+1} states, (t, b) cols
            HT8 = sb.tile([128, KH, NROW], FP8)     # 4x h in fp8 for phase 2

            # warm up the collective path (first CC op pays ~25us extra)
            warm_in = dp.tile([128, 1], F32, tag="warm_in")
            warm_out = dp.tile([128, 1], F32, tag="warm_out",
                               addr_space="Shared")
            nc.gpsimd.dma_start(warm_in[:], ebias[:])
            nc.gpsimd.collective_compute(
                "AllReduce", OP.add, replica_groups=[list(range(NCORES))],
                ins=[warm_in.opt()], outs=[warm_out.opt()])

            emit_prep_xg(0, embt0, range(GC))

            # ---------------- phase 1 step ----------------------------------
            r_off, z_off, n_off = 0, 4, 8   # gate chunk offsets (r, z, n)

            def emit_step(t):
                h_prev = h0_sb[:, :, :] if t == 0 else HT[:, :, (t - 1) * B:t * B]
                xg = xg_tiles[t // 8][:, t % 8, :, :]
                ps_r = ps.tile([128, 4, B], F32, tag="ps_r", name=f"psr{t}")
                ps_n = ps.tile([128, 4, B], F32, tag="ps_n", name=f"psn{t}")
                ps_z = ps.tile([128, 4, B], F32, tag="ps_z", name=f"psz{t}")

                def gate_mms(dst, off, fold):
                    if fold:
                        nc.tensor.matmul(dst[:], lhsT=ident[:],
                                         rhs=xg[:, off:off + 4, :],
                                         start=True, stop=False)
                    for gc in range(4):
                        for kt in range(KH):
                            nc.tensor.matmul(
                                dst[:, gc, :],
                                lhsT=whh_sb[:, kt, (off + gc) * 128:(off + gc + 1) * 128],
                                rhs=h_prev[:, kt, :],
                                start=(not fold and kt == 0),
                                stop=(kt == KH - 1))

                # burst order: r first (binding n-path needs r_s), then n,
                # then z (its consumers have the most slack).
                gate_mms(ps_r, r_off, True)
                gate_mms(ps_n, n_off, False)
                gate_mms(ps_z, z_off, True)

                # r gate: sigma(x) = 0.5*tanh(x/2) + 0.5  (pre-acts are 16x)
                rt = sb.tile([128, 4, B], BF16, tag="rt", bufs=2, name=f"rt{t}")
                nc.scalar.activation(rt[:], ps_r[:], AF.Tanh,
                                     scale=0.5 / WHH_SCALE)
                r_s = sb.tile([128, 4, B], BF16, tag="r_s", bufs=2, name=f"rs{t}")
                nc.vector.tensor_scalar(out=r_s[:], in0=rt[:], scalar1=0.5,
                                        scalar2=0.5, op0=OP.mult, op1=OP.add)
                # z gate
                zt = sb.tile([128, 4, B], BF16, tag="zt", bufs=2, name=f"zt{t}")
                nc.scalar.activation(zt[:], ps_z[:], AF.Tanh,
                                     scale=0.5 / WHH_SCALE)
                q_s = sb.tile([128, 4, B], BF16, tag="q_s", bufs=2, name=f"qs{t}")
                nc.gpsimd.tensor_scalar(out=q_s[:], in0=zt[:], scalar1=-0.5,
                                        scalar2=0.5, op0=OP.mult, op1=OP.add)
                z_s = sb.tile([128, 4, B], BF16, tag="z_s", bufs=2, name=f"zs{t}")
                nc.gpsimd.tensor_scalar(out=z_s[:], in0=zt[:], scalar1=0.5,
                                        scalar2=0.5, op0=OP.mult, op1=OP.add)
                p_s = sb.tile([128, 4, B], BF16, tag="p_s", bufs=2, name=f"ps{t}")
                nc.gpsimd.tensor_tensor(out=p_s[:], in0=z_s[:], in1=h_prev,
                                        op=OP.mult)
                # n gate: tanh((16*xn + r*(16*hn)) / 16)
                if bhh_n_nonzero:
                    nc.vector.tensor_tensor(
                        out=ps_n[:], in0=ps_n[:],
                        in1=bhn_sb[:, :, None].to_broadcast([128, 4, B]), op=OP.add)
                nc.vector.tensor_tensor(out=ps_n[:], in0=ps_n[:], in1=r_s[:],
                                        op=OP.mult)
                nc.vector.tensor_tensor(out=ps_n[:], in0=ps_n[:],
                                        in1=xg[:, n_off:n_off + 4, :], op=OP.add)
                n_s = sb.tile([128, 4, B], BF16, tag="n_s", bufs=2, name=f"ns{t}")
                nc.scalar.activation(n_s[:], ps_n[:], AF.Tanh,
                                     scale=1.0 / WHH_SCALE)
                # h' = n*(1-z) + z*h
                w_s = sb.tile([128, 4, B], BF16, tag="w_s", bufs=2, name=f"ws{t}")
                nc.vector.tensor_tensor(out=w_s[:], in0=n_s[:], in1=q_s[:],
                                        op=OP.mult)
                nc.vector.tensor_tensor(out=HT[:, :, t * B:(t + 1) * B],
                                        in0=w_s[:], in1=p_s[:], op=OP.add)
                # fp8 copy (x4) for phase 2, off the binding path
                eng = nc.gpsimd if ht8_gps else nc.vector
                eng.tensor_scalar(out=HT8[:, :, t * B:(t + 1) * B],
                                  in0=HT[:, :, t * B:(t + 1) * B],
                                  scalar1=HT8_SCALE, scalar2=None,
                                  op0=OP.mult)

            # ---------------- phase 2 emission helpers ----------------------
            logit_tiles = {}
            lse_tiles = {}

            def emit_munit(m, u):
                # one 1000-vocab unit of row-tile m: DoubleRow fp8 matmuls,
                # f16 raw-logit evacuation, exp + row-sum accumulation.
                mp = min(128, NROW - m * 128)
                if u == 0:
                    logit_tiles[m] = sb.tile([128, VS], F16, tag="logit",
                                             bufs=5, name=f"lg{m}")
                lg = logit_tiles[m]
                pl = ps.tile([128, VU], F32, tag="ps_l", bufs=2,
                             name=f"pl{m}_{u}")
                if use_dr:
                    for khi in range(2):
                        for half in range(2):
                            nc.tensor.matmul(
                                pl[:mp, half * 500:(half + 1) * 500],
                                lhsT=HT8[:, 2 * khi:2 * khi + 2,
                                         m * 128:m * 128 + mp],
                                rhs=wpr_sb[:, 2 * khi:2 * khi + 2,
                                           u * VU + half * 500:u * VU + (half + 1) * 500],
                                start=(khi == 0), stop=(khi == 1),
                                perf_mode=DR)
                else:
                    for half in range(2):
                        for kt in range(KH):
                            nc.tensor.matmul(
                                pl[:mp, half * 500:(half + 1) * 500],
                                lhsT=HT8[:, kt, m * 128:m * 128 + mp],
                                rhs=wpr_sb[:, kt,
                                           u * VU + half * 500:u * VU + (half + 1) * 500],
                                start=(kt == 0), stop=(kt == KH - 1))
                if bproj_nonzero:
                    nc.vector.tensor_tensor(
                        out=pl[:mp], in0=pl[:mp],
                        in1=bpr_sb[:mp, u * VU:(u + 1) * VU], op=OP.add)
                nc.vector.tensor_scalar(
                    out=lg[:mp, u * VU:(u + 1) * VU], in0=pl[:mp],
                    scalar1=1.0 / LG_SCALE, scalar2=None, op0=OP.mult)
                esc = sb.tile([128, VU], F16, tag="exps", bufs=2,
                              name=f"esc{m}_{u}")
                nc.scalar.activation(esc[:mp], pl[:mp], AF.Exp,
                                     bias=ebias[:mp, :1], scale=1.0 / LG_SCALE,
                                     accum_out=S_all[:mp, m * NVU + u:m * NVU + u + 1])

            cc_tiles = {}

            def emit_stats_start(g):
                # local row-sums for the group's 2 row-tiles + AllReduce
                sg = sb.tile([128, MPG], F32, tag="sg", bufs=2, name=f"sg{g}")
                for j in range(MPG):
                    m = g * MPG + j
                    nc.vector.reduce_sum(
                        out=sg[:, j:j + 1],
                        in_=S_all[:, m * NVU:(m + 1) * NVU],
                        axis=mybir.AxisListType.X)
                cin = dp.tile([128, MPG], F32, tag=f"cin{g}", name=f"cin{g}")
                nc.gpsimd.dma_start(cin[:], sg[:])
                cout = dp.tile([128, MPG], F32, tag=f"cout{g}",
                               addr_space="Shared", name=f"cout{g}")
                nc.gpsimd.collective_compute(
                    "AllReduce", OP.add,
                    replica_groups=[list(range(NCORES))],
                    ins=[cin.opt()], outs=[cout.opt()])
                cc_tiles[g] = cout

            def emit_stats_consume(g):
                st = sb.tile([128, MPG], F32, tag="st", bufs=2, name=f"st{g}")
                nc.sync.dma_start(st[:], cc_tiles[g][:])
                # neg_lse = -(e - 127 + 4) * ln2 - ln(m),  St = m * 2^(e-127)
                iu = st[:].bitcast(U32)
                eu = sb.tile([128, MPG], U32, tag="eu", bufs=2, name=f"eu{g}")
                nc.vector.tensor_scalar(out=eu[:], in0=iu, scalar1=23,
                                        scalar2=None, op0=OP.logical_shift_right)
                ef = sb.tile([128, MPG], F32, tag="ef", bufs=2, name=f"ef{g}")
                nc.vector.tensor_copy(ef[:], eu[:])
                mu = sb.tile([128, MPG], U32, tag="mu", bufs=2, name=f"mu{g}")
                nc.vector.tensor_scalar(out=mu[:], in0=iu, scalar1=0x007FFFFF,
                                        scalar2=0x3F800000, op0=OP.bitwise_and,
                                        op1=OP.bitwise_or)
                mf = mu[:].bitcast(F32)
                acc = sb.tile([128, MPG], F32, tag="acc", bufs=2, name=f"acc{g}")
                c = _NEGLN_COEF
                nc.vector.tensor_scalar(out=acc[:], in0=mf, scalar1=c[0],
                                        scalar2=c[1], op0=OP.mult, op1=OP.add)
                for k in range(2, 4):
                    nc.vector.tensor_tensor(out=acc[:], in0=acc[:], in1=mf,
                                            op=OP.mult)
                    nc.vector.tensor_scalar(out=acc[:], in0=acc[:], scalar1=c[k],
                                            scalar2=None, op0=OP.add)
                # + (127 - 4 - e) * ln2   (the -4 re-adds the exp bias)
                e2 = sb.tile([128, MPG], F32, tag="e2", bufs=2, name=f"e2{g}")
                nc.vector.tensor_scalar(out=e2[:], in0=ef[:], scalar1=-LN2,
                                        scalar2=(127.0 - 4.0) * LN2,
                                        op0=OP.mult, op1=OP.add)
                nlse = sb.tile([128, MPG], F32, tag="nlse", bufs=2,
                               name=f"nlse{g}")
                nc.vector.tensor_tensor(out=nlse[:], in0=acc[:], in1=e2[:],
                                        op=OP.add)
                lse_tiles[g] = nlse
                if debug and g == 0:
                    nc.sync.dma_start(nlse_d[:], nlse[:])

            def emit_output(m):
                g, j = m // MPG, m % MPG
                mp = min(128, NROW - m * 128)
                nlse = lse_tiles[g]
                lg = logit_tiles.pop(m)
                if debug and m == 0:
                    nc.sync.dma_start(lg_d[:], lg[:])
                ot = sb.tile([128, VS], F16, tag="ot", bufs=2, name=f"ot{m}")
                nc.vector.tensor_tensor(
                    out=ot[:mp], in0=lg[:mp],
                    in1=nlse[:mp, j:j + 1].to_broadcast([mp, VS]), op=OP.add)
                nc.sync.dma_start(out_d[m * 128:m * 128 + mp, :], ot[:mp])

            # ---------------- main emission loop ----------------------------
            from collections import deque
            work_q = deque()

            def enqueue_mtile(m):
                # deferred consume of the group started ~4 steps ago: its
                # AllReduce has been in flight since then.
                if m >= 2 and m % 2 == 0:
                    g = (m - 2) // 2

                    def fin(g=g):
                        emit_stats_consume(g)
                        for mm in range(g * MPG, (g + 1) * MPG):
                            emit_output(mm)
                    work_q.append(fin)
                for u in range(NVU):
                    work_q.append(lambda m=m, u=u: emit_munit(m, u))
                if m % 2 == 1:
                    work_q.append(lambda g=m // 2: emit_stats_start(g))

            for t in range(S):
                emit_step(t)
                if t % 8 == 1 and t // 8 + 1 <= (S - 1) // 8:
                    c8 = t // 8 + 1
                    embt = emit_prep_gather(c8)
                    for lo in range(0, GC, 3):
                        work_q.append(lambda c8=c8, embt=embt, lo=lo:
                                      emit_prep_xg(c8, embt, range(lo, min(lo + 3, GC))))
                if t >= 3 and (t - 3) % 4 == 0:
                    enqueue_mtile((t - 3) // 4)
                for _ in range(min(3, len(work_q))):
                    work_q.popleft()()
            # tail: tile 15, last group stats, remaining consumes/outputs
            enqueue_mtile(15)
            work_q.append(lambda: emit_stats_consume(7))
            work_q.append(lambda: emit_output(14))
            work_q.append(lambda: emit_output(15))
            while work_q:
                work_q.popleft()()
            if debug:
                nc.sync.dma_start(ht_d[:], HT[:])
                nc.sync.dma_start(sall_d[:], S_all[:])

    nc.finalize()
    _BUILD_CACHE[key] = nc
    return nc


def _pack_T(w, ktiles, scale, np_dt):
    """[out, in] f32 -> [128, ktiles, out] (w.T * scale, k-major slabs)."""
    wT = np.ascontiguousarray(w.T) * scale
    return np.ascontiguousarray(
        wT.reshape(ktiles, 128, w.shape[0]).transpose(1, 0, 2)).astype(np_dt)


LAST_PROFILE = None


def kernel(trg, h0, embed_table, W_ih, W_hh, b_ih, b_hh, W_proj, b_proj):
    global LAST_PROFILE
    trg = np.asarray(trg)
    h0 = np.asarray(h0, dtype=np.float32)
    embed_table = np.asarray(embed_table, dtype=np.float32)
    W_ih = np.asarray(W_ih, dtype=np.float32)
    W_hh = np.asarray(W_hh, dtype=np.float32)
    b_ih = np.asarray(b_ih, dtype=np.float32)
    b_hh = np.asarray(b_hh, dtype=np.float32)
    W_proj = np.asarray(W_proj, dtype=np.float32)
    b_proj = np.asarray(b_proj, dtype=np.float32)

    # bx = b_ih + [b_hh for r,z chunks; 0 for n chunks], scaled like xg
    bx = b_ih.copy()
    bx[:2 * H] += b_hh[:2 * H]
    bx_nonzero = bool(np.any(bx))
    bhh_n_nonzero = bool(np.any(b_hh[2 * H:]))
    bproj_nonzero = bool(np.any(b_proj))
    nc = _build(bx_nonzero, bhh_n_nonzero, bproj_nonzero)

    trg_flat = np.ascontiguousarray(
        trg[:, :S].T.reshape(NROW, 1)).astype(np.int32)
    tbl_bf = embed_table.astype(ml_dtypes.bfloat16)
    wih_t = _pack_T(W_ih, KE, WHH_SCALE, ml_dtypes.bfloat16)
    whh_t = _pack_T(W_hh, KH, WHH_SCALE, NP_FP8)
    h0_t = np.ascontiguousarray(
        h0[0].T.reshape(KH, 128, B).transpose(1, 0, 2)).astype(ml_dtypes.bfloat16)

    base = {
        "trg_flat": trg_flat,
        "emb_tbl": tbl_bf,
        "wih_t": wih_t,
        "whh_t": whh_t,
        "h0_t": h0_t,
    }
    if bx_nonzero:
        base["bx_t"] = np.ascontiguousarray(
            (bx * WHH_SCALE).reshape(GC, 128).T).astype(ml_dtypes.bfloat16)
    if bhh_n_nonzero:
        base["bhn_t"] = np.ascontiguousarray(
            (b_hh[2 * H:] * WHH_SCALE).reshape(KH, 128).T).astype(ml_dtypes.bfloat16)

    in_maps = []
    for c in range(NCORES):
        m = dict(base)
        m["wproj_t"] = _pack_T(W_proj[c * VS:(c + 1) * VS], KH, WPR_SCALE,
                               NP_FP8)
        if bproj_nonzero:
            m["bproj_s"] = np.ascontiguousarray(
                (b_proj[c * VS:(c + 1) * VS] * LG_SCALE).reshape(1, VS))
        in_maps.append(m)

    trace = bool(int(os.environ.get("KERNEL_TRACE", "0")))
    res = run_bass_kernel_spmd(nc, in_maps, core_ids=list(range(NCORES)),
                               trace=trace)
    LAST_PROFILE = res

    out = np.zeros((B, T, V), dtype=np.float32)
    big = np.stack([res.results[c]["out_lp"].astype(np.float32).reshape(S, B, VS)
                    for c in range(NCORES)], axis=0)   # [c, t, b, vs]
    out[:, 1:, :] = big.transpose(2, 1, 0, 3).reshape(B, S, V)
    return out


# revision 19
# speedup vs baseline: 1.0499x; 1.0417x over previous
"""GRU decoder (teacher forcing) + log_softmax on 8 Trainium2 NeuronCores.

Strategy (v2):
  - Vocab-shard projection/log-softmax across 8 cores; replicate the serial
    GRU recurrence on every core.
  - Phase 0 (per 8-step chunk): indirect-DMA gather of embedding rows,
    DMA-xbar transpose to k-major, matmul -> x-side gate pre-acts
    XG = 16 * emb @ W_ih.T stored time-major in SBUF (the 16x matches the
    fp8 W_hh scaling so gate pre-acts share one scale).
  - Phase 1 (63 sequential steps): W_hh kept in fp8e4m3 (x16) -> FWL loads
    weight slabs at 2x bf16 rate; per gate the x-side pre-acts are folded
    into PSUM with ONE 128-col identity matmul (start=True) and the W_hh
    matmuls accumulate on top.  Burst order r -> n -> z puts the binding
    n-path dependencies earliest.  sigma(x)=0.5*tanh(x/2)+0.5 so only the
    exp_and_others ACT table is used.  h' lands in bf16 HT and (x4) in fp8
    HT8 for phase 2.
  - Phase 2 (16 row-tiles): DoubleRow fp8 matmuls (HT8 x W_projT8, both
    pre-scaled; logits in PSUM are 256x) in [128,1000] vocab units;
    VectorE evacuates raw logits (/256) to f16; ScalarE exp(l - 4ln2) with
    accum_out collects row sums.  Per 2 row-tiles one tiny AllReduce sums
    the softmax denominators across cores; -lse via DVE frexp + deg-3
    poly; out = logit + (-lse) on DVE in f16, DMA'd out as f16.
  - Collective consumption is deferred ~4 steps behind its start so the
    ~5us CC latency never blocks any engine queue.
  - Startup DMAs are spread across engine queues (sync/vector/scalar) so
    the first GRU step starts ~7us in.

kernel(**inputs) takes FULL numpy inputs, preps layouts on host, runs the
SPMD NEFF on cores 0..7 and reassembles the [32, 64, 32000] f32 output.
"""

import os

import numpy as np
import ml_dtypes

import concourse.bass as bass
import concourse.bacc as bacc
import concourse.mybir as mybir
import concourse.tile as tile
from concourse.bass_utils import run_bass_kernel_spmd
from concourse.masks import make_identity

# problem shape (hardcoded per contract)
B, T, V, E, H = 32, 64, 32000, 256, 512
S = T - 1                 # 63 decode steps
NCORES = 8
VS = V // NCORES          # 4000 vocab shard per core
G = 3 * H                 # 1536 gate dims
GC = G // 128             # 12 gate chunks
KH = H // 128             # 4 contraction tiles over H
KE = E // 128             # 2 contraction tiles over E
NROW = S * B              # 2016 output rows, (t, b) order
NMT = (NROW + 127) // 128  # 16 row-tiles (last has 96 rows)
NGRP = 8                  # stat-collective groups (2 row-tiles each)
MPG = NMT // NGRP         # 2 row-tiles per group
VU = 1000                 # vocab unit for psum/exp
NVU = VS // VU            # 4 units per row-tile
LN2 = float(np.log(2.0))
EXP_BIAS = -4.0 * LN2     # exp(logit - 4ln2): keeps fp16 exp safely in range
WHH_SCALE = 16.0          # fp8 W_hh (and x-gate) pre-scale
HT8_SCALE = 4.0           # fp8 h pre-scale for phase 2
WPR_SCALE = 64.0          # fp8 W_proj pre-scale
LG_SCALE = HT8_SCALE * WPR_SCALE   # phase-2 PSUM logits are 256x

F32 = mybir.dt.float32
BF16 = mybir.dt.bfloat16
F16 = mybir.dt.float16
FP8 = mybir.dt.float8e4
I32 = mybir.dt.int32
U32 = mybir.dt.uint32
AF = mybir.ActivationFunctionType
OP = mybir.AluOpType
DR = mybir.MatmulPerfMode.DoubleRow
NP_FP8 = ml_dtypes.float8_e4m3fn

# -ln(m) Chebyshev-interpolation coefficients on m in [1, 2], highest first.
_nodes = np.cos((2 * np.arange(1, 5) - 1) / (2 * 4.0) * np.pi) * 0.5 + 1.5
_NEGLN_COEF = [float(c) for c in np.polyfit(_nodes, -np.log(_nodes), 3)]

_BUILD_CACHE = {}


def _build(bx_nonzero: bool, bhh_n_nonzero: bool, bproj_nonzero: bool):
    debug = bool(int(os.environ.get("KERNEL_DEBUG", "0")))
    use_dr = bool(int(os.environ.get("KERNEL_DR", "0")))
    ht8_gps = bool(int(os.environ.get("KERNEL_HT8GPS", "0")))
    key = (bx_nonzero, bhh_n_nonzero, bproj_nonzero, debug, use_dr, ht8_gps)
    if key in _BUILD_CACHE:
        return _BUILD_CACHE[key]

    nc = bacc.Bacc("TRN2", target_bir_lowering=False, debug=False,
                   enable_asserts=False, num_devices=NCORES)

    trg_d = nc.dram_tensor("trg_flat", (NROW, 1), I32, kind="ExternalInput")
    tbl_d = nc.dram_tensor("emb_tbl", (V, E), BF16, kind="ExternalInput")
    wih_d = nc.dram_tensor("wih_t", (128, KE, G), BF16, kind="ExternalInput")
    whh_d = nc.dram_tensor("whh_t", (128, KH, G), FP8, kind="ExternalInput")
    h0_d = nc.dram_tensor("h0_t", (128, KH, B), BF16, kind="ExternalInput")
    wpr_d = nc.dram_tensor("wproj_t", (128, KH, VS), BF16, kind="ExternalInput")
    if bx_nonzero:
        bx_d = nc.dram_tensor("bx_t", (128, GC), BF16, kind="ExternalInput")
    if bhh_n_nonzero:
        bhn_d = nc.dram_tensor("bhn_t", (128, KH), BF16, kind="ExternalInput")
    if bproj_nonzero:
        bpr_d = nc.dram_tensor("bproj_s", (1, VS), F32, kind="ExternalInput")
    out_d = nc.dram_tensor("out_lp", (NROW, VS), F16, kind="ExternalOutput")
    if debug:
        ht_d = nc.dram_tensor("dbg_ht", (128, KH, NROW), BF16,
                              kind="ExternalOutput")
        sall_d = nc.dram_tensor("dbg_sall", (128, NMT * NVU), F32,
                                kind="ExternalOutput")
        lg_d = nc.dram_tensor("dbg_lg", (128, VS), F16, kind="ExternalOutput")
        nlse_d = nc.dram_tensor("dbg_nlse", (128, MPG), F32,
                                kind="ExternalOutput")

    with tile.TileContext(nc) as tc:
        with tc.tile_pool(name="sb", bufs=1) as sb, \
             tc.tile_pool(name="ps", bufs=1, space="PSUM") as ps, \
             tc.tile_pool(name="dram", bufs=1, space="DRAM") as dp:

            # ---------- phase-0 prep for chunk 0 first (critical path) ------
            ebias = sb.tile([128, 1], F32)
            nc.gpsimd.memset(ebias[:], EXP_BIAS)
            ident = sb.tile([128, 128], BF16)
            make_identity(nc, ident[:])
            S_all = sb.tile([128, NMT * NVU], F32)   # exp partial sums
            nc.gpsimd.memset(S_all[:], 0.0)

            # warm up the collective path first (cold CC init is ~25us); the
            # trigger itself is non-blocking on the gpsimd queue.
            warm_in = dp.tile([128, 1], F32, tag="warm_in")
            warm_out = dp.tile([128, 1], F32, tag="warm_out",
                               addr_space="Shared")
            nc.gpsimd.dma_start(warm_in[:], ebias[:])
            nc.gpsimd.collective_compute(
                "AllReduce", OP.add, replica_groups=[list(range(NCORES))],
                ins=[warm_in.opt()], outs=[warm_out.opt()])

            xg_tiles = {}

            def emit_prep_gather(c8):
                tlo = 8 * c8
                nst = min(8, S - tlo)
                nrows = B * nst
                xg = sb.tile([128, 8, GC, B], BF16, tag="xg", bufs=2,
                             name=f"xg{c8}")
                xg_tiles[c8] = xg
                embt = sb.tile([128, KE, 256], BF16, tag="embt", bufs=2,
                               name=f"embt{c8}")
                for sub in range(2):
                    lo = tlo * B + sub * 128
                    nr = min(128, nrows - sub * 128)
                    if nr <= 0:
                        continue
                    idx_t = sb.tile([128, 1], I32, tag="idx", bufs=4,
                                    name=f"idx{c8}_{sub}")
                    nc.sync.dma_start(idx_t[:nr], trg_d[lo:lo + nr, :])
                    rows = sb.tile([128, E], BF16, tag="embr", bufs=4,
                                   name=f"embr{c8}_{sub}")
                    nc.gpsimd.indirect_dma_start(
                        out=rows[:nr], out_offset=None, in_=tbl_d[:],
                        in_offset=bass.IndirectOffsetOnAxis(ap=idx_t[:nr, :1], axis=0))
                    for kb in range(KE):
                        nc.sync.dma_start_transpose(
                            embt[:, kb, sub * 128:sub * 128 + nr],
                            rows[:nr, kb * 128:(kb + 1) * 128])
                return embt

            def emit_prep_xg(c8, embt, gcs):
                tlo = 8 * c8
                nst = min(8, S - tlo)
                nrows = B * nst
                xg = xg_tiles[c8]
                for gc in gcs:
                    pxg = ps.tile([128, 256], F32, tag="ps_xg", bufs=1,
                                  name=f"pxg{c8}_{gc}")
                    for kt in range(KE):
                        nc.tensor.matmul(
                            pxg[:, :nrows],
                            lhsT=wih_sb[:, kt, gc * 128:(gc + 1) * 128],
                            rhs=embt[:, kt, :nrows],
                            start=(kt == 0), stop=(kt == KE - 1))
                    src = pxg[:, :nrows].rearrange("p (t b) -> p t b", b=B)
                    if bx_nonzero:
                        nc.vector.tensor_tensor(
                            out=xg[:, :nst, gc, :], in0=src,
                            in1=bx_sb[:, gc:gc + 1].to_broadcast([128, nst, B]),
                            op=OP.add)
                    else:
                        nc.scalar.activation(xg[:, :nst, gc, :], src, AF.Copy)

            embt0 = emit_prep_gather(0)

            # ---------- persistent loads spread over engine DMA queues ------
            wih_sb = sb.tile([128, KE, G], BF16)
            nc.scalar.dma_start(wih_sb[:], wih_d[:])
            whh_sb = sb.tile([128, KH, G], FP8)
            nc.scalar.dma_start(whh_sb[:], whh_d[:])
            h0_sb = sb.tile([128, KH, B], BF16)
            nc.scalar.dma_start(h0_sb[:], h0_d[:])
            wpr_sb = sb.tile([128, KH, VS], BF16)
            nc.scalar.dma_start(wpr_sb[:], wpr_d[:])
            if bx_nonzero:
                bx_sb = sb.tile([128, GC], BF16)
                nc.gpsimd.dma_start(bx_sb[:], bx_d[:])
            if bhh_n_nonzero:
                bhn_sb = sb.tile([128, KH], BF16)
                nc.gpsimd.dma_start(bhn_sb[:], bhn_d[:])
            if bproj_nonzero:
                bpr_sb = sb.tile([128, VS], F32)
                nc.gpsimd.dma_start(bpr_sb[:], bpr_d[:1, :].to_broadcast([128, VS]))

            HT = sb.tile([128, KH, NROW], BF16)     # h_{t+1} states, (t, b) cols
            HT8 = sb.tile([128, KH, NROW], FP8)     # 4x h in fp8 for phase 2

            emit_prep_xg(0, embt0, range(GC))

            # ---------------- phase 1 step ----------------------------------
            r_off, z_off, n_off = 0, 4, 8   # gate chunk offsets (r, z, n)

            def emit_step(t):
                h_prev = h0_sb[:, :, :] if t == 0 else HT[:, :, (t - 1) * B:t * B]
                xg = xg_tiles[t // 8][:, t % 8, :, :]
                ps_r = ps.tile([128, 4, B], F32, tag="ps_r", name=f"psr{t}")
                ps_n = ps.tile([128, 4, B], F32, tag="ps_n", name=f"psn{t}")
                ps_z = ps.tile([128, 4, B], F32, tag="ps_z", name=f"psz{t}")

                def gate_mms(dst, off, fold):
                    if fold:
                        nc.tensor.matmul(dst[:], lhsT=ident[:],
                                         rhs=xg[:, off:off + 4, :],
                                         start=True, stop=False)
                    for gc in range(4):
                        for kt in range(KH):
                            nc.tensor.matmul(
                                dst[:, gc, :],
                                lhsT=whh_sb[:, kt, (off + gc) * 128:(off + gc + 1) * 128],
                                rhs=h_prev[:, kt, :],
                                start=(not fold and kt == 0),
                                stop=(kt == KH - 1))

                # burst order: r first (binding n-path needs r_s), then n,
                # then z (its consumers have the most slack).
                gate_mms(ps_r, r_off, True)
                gate_mms(ps_n, n_off, False)
                gate_mms(ps_z, z_off, True)

                # r gate: sigma(x) = 0.5*tanh(x/2) + 0.5  (pre-acts are 16x)
                rt = sb.tile([128, 4, B], BF16, tag="rt", bufs=2, name=f"rt{t}")
                nc.scalar.activation(rt[:], ps_r[:], AF.Tanh,
                                     scale=0.5 / WHH_SCALE)
                r_s = sb.tile([128, 4, B], BF16, tag="r_s", bufs=2, name=f"rs{t}")
                nc.vector.tensor_scalar(out=r_s[:], in0=rt[:], scalar1=0.5,
                                        scalar2=0.5, op0=OP.mult, op1=OP.add)
                # z gate
                zt = sb.tile([128, 4, B], BF16, tag="zt", bufs=2, name=f"zt{t}")
                nc.scalar.activation(zt[:], ps_z[:], AF.Tanh,
                                     scale=0.5 / WHH_SCALE)
                q_s = sb.tile([128, 4, B], BF16, tag="q_s", bufs=2, name=f"qs{t}")
                nc.gpsimd.tensor_scalar(out=q_s[:], in0=zt[:], scalar1=-0.5,
                                        scalar2=0.5, op0=OP.mult, op1=OP.add)
                z_s = sb.tile([128, 4, B], BF16, tag="z_s", bufs=2, name=f"zs{t}")
                nc.gpsimd.tensor_scalar(out=z_s[:], in0=zt[:], scalar1=0.5,
                                        scalar2=0.5, op0=OP.mult, op1=OP.add)
                p_s = sb.tile([128, 4, B], BF16, tag="p_s", bufs=2, name=f"ps{t}")
                nc.gpsimd.tensor_tensor(out=p_s[:], in0=z_s[:], in1=h_prev,
                                        op=OP.mult)
                # n gate: tanh((16*xn + r*(16*hn)) / 16)
                if bhh_n_nonzero:
                    nc.vector.tensor_tensor(
                        out=ps_n[:], in0=ps_n[:],
                        in1=bhn_sb[:, :, None].to_broadcast([128, 4, B]), op=OP.add)
                nc.vector.tensor_tensor(out=ps_n[:], in0=ps_n[:], in1=r_s[:],
                                        op=OP.mult)
                nc.vector.tensor_tensor(out=ps_n[:], in0=ps_n[:],
                                        in1=xg[:, n_off:n_off + 4, :], op=OP.add)
                n_s = sb.tile([128, 4, B], BF16, tag="n_s", bufs=2, name=f"ns{t}")
                nc.scalar.activation(n_s[:], ps_n[:], AF.Tanh,
                                     scale=1.0 / WHH_SCALE)
                # h' = n*(1-z) + z*h
                w_s = sb.tile([128, 4, B], BF16, tag="w_s", bufs=2, name=f"ws{t}")
                nc.vector.tensor_tensor(out=w_s[:], in0=n_s[:], in1=q_s[:],
                                        op=OP.mult)
                nc.vector.tensor_tensor(out=HT[:, :, t * B:(t + 1) * B],
                                        in0=w_s[:], in1=p_s[:], op=OP.add)
                # fp8 copy (x4) for phase 2, off the binding path
                eng = nc.gpsimd if ht8_gps else nc.vector
                eng.tensor_scalar(out=HT8[:, :, t * B:(t + 1) * B],
                                  in0=HT[:, :, t * B:(t + 1) * B],
                                  scalar1=HT8_SCALE, scalar2=None,
                                  op0=OP.mult)

            # ---------------- phase 2 emission helpers ----------------------
            logit_tiles = {}
            lse_tiles = {}

            def emit_munit(m, u):
                # one 1000-vocab unit of row-tile m: DoubleRow fp8 matmuls,
                # f16 raw-logit evacuation, exp + row-sum accumulation.
                mp = min(128, NROW - m * 128)
                if u == 0:
                    logit_tiles[m] = sb.tile([128, VS], F16, tag="logit",
                                             bufs=5, name=f"lg{m}")
                lg = logit_tiles[m]
                pl = ps.tile([128, VU], F32, tag="ps_l", bufs=2,
                             name=f"pl{m}_{u}")
                if use_dr:
                    for khi in range(2):
                        for half in range(2):
                            nc.tensor.matmul(
                                pl[:mp, half * 500:(half + 1) * 500],
                                lhsT=HT8[:, 2 * khi:2 * khi + 2,
                                         m * 128:m * 128 + mp],
                                rhs=wpr_sb[:, 2 * khi:2 * khi + 2,
                                           u * VU + half * 500:u * VU + (half + 1) * 500],
                                start=(khi == 0), stop=(khi == 1),
                                perf_mode=DR)
                else:
                    for half in range(2):
                        for kt in range(KH):
                            nc.tensor.matmul(
                                pl[:mp, half * 500:(half + 1) * 500],
                                lhsT=HT8[:, kt, m * 128:m * 128 + mp],
                                rhs=wpr_sb[:, kt,
                                           u * VU + half * 500:u * VU + (half + 1) * 500],
                                start=(kt == 0), stop=(kt == KH - 1))
                if bproj_nonzero:
                    nc.vector.tensor_tensor(
                        out=pl[:mp], in0=pl[:mp],
                        in1=bpr_sb[:mp, u * VU:(u + 1) * VU], op=OP.add)
                nc.vector.tensor_scalar(
                    out=lg[:mp, u * VU:(u + 1) * VU], in0=pl[:mp],
                    scalar1=1.0 / LG_SCALE, scalar2=None, op0=OP.mult)
                esc = sb.tile([128, VU], F16, tag="exps", bufs=2,
                              name=f"esc{m}_{u}")
                nc.scalar.activation(esc[:mp], pl[:mp], AF.Exp,
                                     bias=ebias[:mp, :1], scale=1.0 / LG_SCALE,
                                     accum_out=S_all[:mp, m * NVU + u:m * NVU + u + 1])

            cc_tiles = {}

            def emit_stats_start(g):
                # local row-sums for the group's 2 row-tiles + AllReduce
                sg = sb.tile([128, MPG], F32, tag="sg", bufs=2, name=f"sg{g}")
                for j in range(MPG):
                    m = g * MPG + j
                    nc.vector.reduce_sum(
                        out=sg[:, j:j + 1],
                        in_=S_all[:, m * NVU:(m + 1) * NVU],
                        axis=mybir.AxisListType.X)
                cin = dp.tile([128, MPG], F32, tag=f"cin{g}", name=f"cin{g}")
                nc.gpsimd.dma_start(cin[:], sg[:])
                cout = dp.tile([128, MPG], F32, tag=f"cout{g}",
                               addr_space="Shared", name=f"cout{g}")
                nc.gpsimd.collective_compute(
                    "AllReduce", OP.add,
                    replica_groups=[list(range(NCORES))],
                    ins=[cin.opt()], outs=[cout.opt()])
                cc_tiles[g] = cout

            def emit_stats_consume(g):
                st = sb.tile([128, MPG], F32, tag="st", bufs=2, name=f"st{g}")
                nc.sync.dma_start(st[:], cc_tiles[g][:])
                # neg_lse = -(e - 127 + 4) * ln2 - ln(m),  St = m * 2^(e-127)
                iu = st[:].bitcast(U32)
                eu = sb.tile([128, MPG], U32, tag="eu", bufs=2, name=f"eu{g}")
                nc.vector.tensor_scalar(out=eu[:], in0=iu, scalar1=23,
                                        scalar2=None, op0=OP.logical_shift_right)
                ef = sb.tile([128, MPG], F32, tag="ef", bufs=2, name=f"ef{g}")
                nc.vector.tensor_copy(ef[:], eu[:])
                mu = sb.tile([128, MPG], U32, tag="mu", bufs=2, name=f"mu{g}")
                nc.vector.tensor_scalar(out=mu[:], in0=iu, scalar1=0x007FFFFF,
                                        scalar2=0x3F800000, op0=OP.bitwise_and,
                                        op1=OP.bitwise_or)
                mf = mu[:].bitcast(F32)
                acc = sb.tile([128, MPG], F32, tag="acc", bufs=2, name=f"acc{g}")
                c = _NEGLN_COEF
                nc.vector.tensor_scalar(out=acc[:], in0=mf, scalar1=c[0],
                                        scalar2=c[1], op0=OP.mult, op1=OP.add)
                for k in range(2, 4):
                    nc.vector.tensor_tensor(out=acc[:], in0=acc[:], in1=mf,
                                            op=OP.mult)
                    nc.vector.tensor_scalar(out=acc[:], in0=acc[:], scalar1=c[k],
                                            scalar2=None, op0=OP.add)
                # + (127 - 4 - e) * ln2   (the -4 re-adds the exp bias)
                e2 = sb.tile([128, MPG], F32, tag="e2", bufs=2, name=f"e2{g}")
                nc.vector.tensor_scalar(out=e2[:], in0=ef[:], scalar1=-LN2,
                                        scalar2=(127.0 - 4.0) * LN2,
                                        op0=OP.mult, op1=OP.add)
                nlse = sb.tile([128, MPG], F32, tag="nlse", bufs=2,
                               name=f"nlse{g}")
                nc.vector.tensor_tensor(out=nlse[:], in0=acc[:], in1=e2[:],
                                        op=OP.add)
                lse_tiles[g] = nlse
                if debug and g == 0:
                    nc.sync.dma_start(nlse_d[:], nlse[:])

            def emit_output(m):
                g, j = m // MPG, m % MPG
                mp = min(128, NROW - m * 128)
                nlse = lse_tiles[g]
                lg = logit_tiles.pop(m)
                if debug and m == 0:
                    nc.sync.dma_start(lg_d[:], lg[:])
                ot = sb.tile([128, VS], F16, tag="ot", bufs=2, name=f"ot{m}")
                # split the +(-lse) pass: half on DVE, half on ACT (per-
                # partition bias) so neither engine eats the full 4000 cols.
                hv = VS // 2
                nc.vector.tensor_tensor(
                    out=ot[:mp, :hv], in0=lg[:mp, :hv],
                    in1=nlse[:mp, j:j + 1].to_broadcast([mp, hv]), op=OP.add)
                nc.scalar.activation(ot[:mp, hv:], lg[:mp, hv:], AF.Identity,
                                     bias=nlse[:mp, j:j + 1])
                nc.sync.dma_start(out_d[m * 128:m * 128 + mp, :], ot[:mp])

            # ---------------- main emission loop ----------------------------
            from collections import deque
            work_q = deque()

            def enqueue_mtile(m):
                # deferred consume of the group started ~4 steps ago: its
                # AllReduce has been in flight since then.
                if m >= 2 and m % 2 == 0:
                    g = (m - 2) // 2

                    def fin(g=g):
                        emit_stats_consume(g)
                        for mm in range(g * MPG, (g + 1) * MPG):
                            emit_output(mm)
                    work_q.append(fin)
                for u in range(NVU):
                    work_q.append(lambda m=m, u=u: emit_munit(m, u))
                if m % 2 == 1:
                    work_q.append(lambda g=m // 2: emit_stats_start(g))

            for t in range(S):
                emit_step(t)
                if t % 8 == 1 and t // 8 + 1 <= (S - 1) // 8:
                    c8 = t // 8 + 1
                    embt = emit_prep_gather(c8)
                    for lo in range(0, GC, 3):
                        work_q.append(lambda c8=c8, embt=embt, lo=lo:
                                      emit_prep_xg(c8, embt, range(lo, min(lo + 3, GC))))
                if t >= 3 and (t - 3) % 4 == 0:
                    enqueue_mtile((t - 3) // 4)
                for _ in range(min(3, len(work_q))):
                    work_q.popleft()()
            # tail: tile 15, last group stats, remaining consumes/outputs
            enqueue_mtile(15)
            work_q.append(lambda: emit_stats_consume(7))
            work_q.append(lambda: emit_output(14))
            work_q.append(lambda: emit_output(15))
            while work_q:
                work_q.popleft()()
            if debug:
                nc.sync.dma_start(ht_d[:], HT[:])
                nc.sync.dma_start(sall_d[:], S_all[:])

    nc.finalize()
    _BUILD_CACHE[key] = nc
    return nc


def _pack_T(w, ktiles, scale, np_dt):
    """[out, in] f32 -> [128, ktiles, out] (w.T * scale, k-major slabs)."""
    wT = np.ascontiguousarray(w.T) * scale
    return np.ascontiguousarray(
        wT.reshape(ktiles, 128, w.shape[0]).transpose(1, 0, 2)).astype(np_dt)


LAST_PROFILE = None


def kernel(trg, h0, embed_table, W_ih, W_hh, b_ih, b_hh, W_proj, b_proj):
    global LAST_PROFILE
    trg = np.asarray(trg)
    h0 = np.asarray(h0, dtype=np.float32)
    embed_table = np.asarray(embed_table, dtype=np.float32)
    W_ih = np.asarray(W_ih, dtype=np.float32)
    W_hh = np.asarray(W_hh, dtype=np.float32)
    b_ih = np.asarray(b_ih, dtype=np.float32)
    b_hh = np.asarray(b_hh, dtype=np.float32)
    W_proj = np.asarray(W_proj, dtype=np.float32)
    b_proj = np.asarray(b_proj, dtype=np.float32)

    # bx = b_ih + [b_hh for r,z chunks; 0 for n chunks], scaled like xg
    bx = b_ih.copy()
    bx[:2 * H] += b_hh[:2 * H]
    bx_nonzero = bool(np.any(bx))
    bhh_n_nonzero = bool(np.any(b_hh[2 * H:]))
    bproj_nonzero = bool(np.any(b_proj))
    nc = _build(bx_nonzero, bhh_n_nonzero, bproj_nonzero)

    trg_flat = np.ascontiguousarray(
        trg[:, :S].T.reshape(NROW, 1)).astype(np.int32)
    tbl_bf = embed_table.astype(ml_dtypes.bfloat16)
    wih_t = _pack_T(W_ih, KE, WHH_SCALE, ml_dtypes.bfloat16)
    whh_t = _pack_T(W_hh, KH, WHH_SCALE, NP_FP8)
    h0_t = np.ascontiguousarray(
        h0[0].T.reshape(KH, 128, B).transpose(1, 0, 2)).astype(ml_dtypes.bfloat16)

    base = {
        "trg_flat": trg_flat,
        "emb_tbl": tbl_bf,
        "wih_t": wih_t,
        "whh_t": whh_t,
        "h0_t": h0_t,
    }
    if bx_nonzero:
        base["bx_t"] = np.ascontiguousarray(
            (bx * WHH_SCALE).reshape(GC, 128).T).astype(ml_dtypes.bfloat16)
    if bhh_n_nonzero:
        base["bhn_t"] = np.ascontiguousarray(
            (b_hh[2 * H:] * WHH_SCALE).reshape(KH, 128).T).astype(ml_dtypes.bfloat16)

    in_maps = []
    for c in range(NCORES):
        m = dict(base)
        m["wproj_t"] = _pack_T(W_proj[c * VS:(c + 1) * VS], KH, WPR_SCALE,
                               ml_dtypes.bfloat16)
        if bproj_nonzero:
            m["bproj_s"] = np.ascontiguousarray(
                (b_proj[c * VS:(c + 1) * VS] * LG_SCALE).reshape(1, VS))
        in_maps.append(m)

    trace = bool(int(os.environ.get("KERNEL_TRACE", "0")))
    res = run_bass_kernel_spmd(nc, in_maps, core_ids=list(range(NCORES)),
                               trace=trace)
    LAST_PROFILE = res

    out = np.zeros((B, T, V), dtype=np.float32)
    big = np.stack([res.results[c]["out_lp"].astype(np.float32).reshape(S, B, VS)
                    for c in range(NCORES)], axis=0)   # [c, t, b, vs]
    out[:, 1:, :] = big.transpose(2, 1, 0, 3).reshape(B, S, V)
    return out


# revision 25
# speedup vs baseline: 1.1106x; 1.0578x over previous
"""GRU decoder (teacher forcing) + log_softmax on 8 Trainium2 NeuronCores.

Strategy (v2):
  - Vocab-shard projection/log-softmax across 8 cores; replicate the serial
    GRU recurrence on every core.
  - Phase 0 (per 8-step chunk): indirect-DMA gather of embedding rows,
    DMA-xbar transpose to k-major, matmul -> x-side gate pre-acts
    XG = 16 * emb @ W_ih.T stored time-major in SBUF (the 16x matches the
    fp8 W_hh scaling so gate pre-acts share one scale).
  - Phase 1 (63 sequential steps): W_hh kept in fp8e4m3 (x16) -> FWL loads
    weight slabs at 2x bf16 rate; per gate the x-side pre-acts are folded
    into PSUM with ONE 128-col identity matmul (start=True) and the W_hh
    matmuls accumulate on top.  Burst order r -> n -> z puts the binding
    n-path dependencies earliest.  sigma(x)=0.5*tanh(x/2)+0.5 so only the
    exp_and_others ACT table is used.  h' lands in bf16 HT and (x4) in fp8
    HT8 for phase 2.
  - Phase 2 (16 row-tiles): DoubleRow fp8 matmuls (HT8 x W_projT8, both
    pre-scaled; logits in PSUM are 256x) in [128,1000] vocab units;
    VectorE evacuates raw logits (/256) to f16; ScalarE exp(l - 4ln2) with
    accum_out collects row sums.  Per 2 row-tiles one tiny AllReduce sums
    the softmax denominators across cores; -lse via DVE frexp + deg-3
    poly; out = logit + (-lse) on DVE in f16, DMA'd out as f16.
  - Collective consumption is deferred ~4 steps behind its start so the
    ~5us CC latency never blocks any engine queue.
  - Startup DMAs are spread across engine queues (sync/vector/scalar) so
    the first GRU step starts ~7us in.

kernel(**inputs) takes FULL numpy inputs, preps layouts on host, runs the
SPMD NEFF on cores 0..7 and reassembles the [32, 64, 32000] f32 output.
"""

import os

import numpy as np
import ml_dtypes

import concourse.bass as bass
import concourse.bacc as bacc
import concourse.mybir as mybir
import concourse.tile as tile
from concourse.bass_utils import run_bass_kernel_spmd
from concourse.masks import make_identity

# problem shape (hardcoded per contract)
B, T, V, E, H = 32, 64, 32000, 256, 512
S = T - 1                 # 63 decode steps
NCORES = 8
VS = V // NCORES          # 4000 vocab shard per core
G = 3 * H                 # 1536 gate dims
GC = G // 128             # 12 gate chunks
KH = H // 128             # 4 contraction tiles over H
KE = E // 128             # 2 contraction tiles over E
NROW = S * B              # 2016 output rows, (t, b) order
NMT = (NROW + 127) // 128  # 16 row-tiles (last has 96 rows)
NGRP = 8                  # stat-collective groups (2 row-tiles each)
MPG = NMT // NGRP         # 2 row-tiles per group
VU = 1000                 # vocab unit for psum/exp
NVU = VS // VU            # 4 units per row-tile
LN2 = float(np.log(2.0))
EXP_BIAS = -4.0 * LN2     # exp(logit - 4ln2): keeps fp16 exp safely in range
WHH_SCALE = 16.0          # fp8 W_hh (and x-gate) pre-scale
HT8_SCALE = 4.0           # fp8 h pre-scale for phase 2
WPR_SCALE = 64.0          # fp8 W_proj pre-scale
LG_SCALE = HT8_SCALE * WPR_SCALE   # phase-2 PSUM logits are 256x

F32 = mybir.dt.float32
BF16 = mybir.dt.bfloat16
F16 = mybir.dt.float16
FP8 = mybir.dt.float8e4
I32 = mybir.dt.int32
U32 = mybir.dt.uint32
AF = mybir.ActivationFunctionType
OP = mybir.AluOpType
DR = mybir.MatmulPerfMode.DoubleRow
NP_FP8 = ml_dtypes.float8_e4m3fn

# -ln(m) Chebyshev-interpolation coefficients on m in [1, 2], highest first.
_nodes = np.cos((2 * np.arange(1, 5) - 1) / (2 * 4.0) * np.pi) * 0.5 + 1.5
_NEGLN_COEF = [float(c) for c in np.polyfit(_nodes, -np.log(_nodes), 3)]

_BUILD_CACHE = {}


def _build(bx_nonzero: bool, bhh_n_nonzero: bool, bproj_nonzero: bool):
    debug = bool(int(os.environ.get("KERNEL_DEBUG", "0")))
    use_dr = bool(int(os.environ.get("KERNEL_DR", "0")))
    ht8_gps = bool(int(os.environ.get("KERNEL_HT8GPS", "0")))
    key = (bx_nonzero, bhh_n_nonzero, bproj_nonzero, debug, use_dr, ht8_gps)
    if key in _BUILD_CACHE:
        return _BUILD_CACHE[key]

    nc = bacc.Bacc("TRN2", target_bir_lowering=False, debug=False,
                   enable_asserts=False, num_devices=NCORES)

    trg_d = nc.dram_tensor("trg_flat", (NROW, 1), I32, kind="ExternalInput")
    tbl_d = nc.dram_tensor("emb_tbl", (V, E), BF16, kind="ExternalInput")
    wih_d = nc.dram_tensor("wih_t", (128, KE, G), BF16, kind="ExternalInput")
    whh_d = nc.dram_tensor("whh_t", (128, KH, G), FP8, kind="ExternalInput")
    h0_d = nc.dram_tensor("h0_t", (128, KH, B), BF16, kind="ExternalInput")
    wpr_d = nc.dram_tensor("wproj_t", (128, KH, VS), BF16, kind="ExternalInput")
    if bx_nonzero:
        bx_d = nc.dram_tensor("bx_t", (128, GC), BF16, kind="ExternalInput")
    if bhh_n_nonzero:
        bhn_d = nc.dram_tensor("bhn_t", (128, KH), BF16, kind="ExternalInput")
    if bproj_nonzero:
        bpr_d = nc.dram_tensor("bproj_s", (1, VS), F32, kind="ExternalInput")
    out_d = nc.dram_tensor("out_lp", (NROW, VS), F16, kind="ExternalOutput")
    if debug:
        ht_d = nc.dram_tensor("dbg_ht", (128, KH, NROW), BF16,
                              kind="ExternalOutput")
        sall_d = nc.dram_tensor("dbg_sall", (128, NMT * NVU), F32,
                                kind="ExternalOutput")
        lg_d = nc.dram_tensor("dbg_lg", (128, VS), F16, kind="ExternalOutput")
        nlse_d = nc.dram_tensor("dbg_nlse", (128, MPG), F32,
                                kind="ExternalOutput")

    with tile.TileContext(nc) as tc:
        with tc.tile_pool(name="sb", bufs=1) as sb, \
             tc.tile_pool(name="ps", bufs=1, space="PSUM") as ps, \
             tc.tile_pool(name="dram", bufs=1, space="DRAM") as dp:

            # ---------- phase-0 prep for chunk 0 first (critical path) ------
            ebias = sb.tile([128, 1], F32)
            nc.gpsimd.memset(ebias[:], EXP_BIAS)
            ident = sb.tile([128, 128], BF16)
            make_identity(nc, ident[:])
            S_all = sb.tile([128, NMT * NVU], F32)   # exp partial sums
            nc.gpsimd.memset(S_all[:], 0.0)



            xg_tiles = {}

            def emit_prep_gather(c8):
                tlo = 8 * c8
                nst = min(8, S - tlo)
                nrows = B * nst
                xg = sb.tile([128, 8, GC, B], BF16, tag="xg", bufs=2,
                             name=f"xg{c8}")
                xg_tiles[c8] = xg
                embt = sb.tile([128, KE, 256], BF16, tag="embt", bufs=2,
                               name=f"embt{c8}")
                for sub in range(2):
                    lo = tlo * B + sub * 128
                    nr = min(128, nrows - sub * 128)
                    if nr <= 0:
                        continue
                    idx_t = sb.tile([128, 1], I32, tag="idx", bufs=4,
                                    name=f"idx{c8}_{sub}")
                    nc.sync.dma_start(idx_t[:nr], trg_d[lo:lo + nr, :])
                    rows = sb.tile([128, E], BF16, tag="embr", bufs=4,
                                   name=f"embr{c8}_{sub}")
                    nc.gpsimd.indirect_dma_start(
                        out=rows[:nr], out_offset=None, in_=tbl_d[:],
                        in_offset=bass.IndirectOffsetOnAxis(ap=idx_t[:nr, :1], axis=0))
                    for kb in range(KE):
                        nc.sync.dma_start_transpose(
                            embt[:, kb, sub * 128:sub * 128 + nr],
                            rows[:nr, kb * 128:(kb + 1) * 128])
                return embt

            def emit_prep_xg(c8, embt, gcs):
                tlo = 8 * c8
                nst = min(8, S - tlo)
                nrows = B * nst
                xg = xg_tiles[c8]
                for g0 in gcs:   # g0 = first of a pair of gate chunks
                    pxg = ps.tile([128, 2, 256], F32, tag="ps_xg", bufs=1,
                                  name=f"pxg{c8}_{g0}")
                    for gi in range(2):
                        for kt in range(KE):
                            nc.tensor.matmul(
                                pxg[:, gi, :nrows],
                                lhsT=wih_sb[:, kt, (g0 + gi) * 128:(g0 + gi + 1) * 128],
                                rhs=embt[:, kt, :nrows],
                                start=(kt == 0), stop=(kt == KE - 1))
                    src = pxg[:, :, :nrows].rearrange("p g (t b) -> p t g b", b=B)
                    if bx_nonzero:
                        for gi in range(2):
                            nc.vector.tensor_tensor(
                                out=xg[:, :nst, g0 + gi, :],
                                in0=pxg[:, gi, :nrows].rearrange(
                                    "p (t b) -> p t b", b=B),
                                in1=bx_sb[:, g0 + gi:g0 + gi + 1].to_broadcast(
                                    [128, nst, B]),
                                op=OP.add)
                    else:
                        nc.vector.tensor_copy(xg[:, :nst, g0:g0 + 2, :], src)

            embt0 = emit_prep_gather(0)

            # ---------- persistent loads spread over engine DMA queues ------
            wih_sb = sb.tile([128, KE, G], BF16)
            nc.scalar.dma_start(wih_sb[:], wih_d[:])
            whh_sb = sb.tile([128, KH, G], FP8)
            nc.scalar.dma_start(whh_sb[:], whh_d[:])
            h0_sb = sb.tile([128, KH, B], BF16)
            nc.scalar.dma_start(h0_sb[:], h0_d[:])
            wpr_sb = sb.tile([128, KH, VS], BF16)
            nc.scalar.dma_start(wpr_sb[:], wpr_d[:])
            if bx_nonzero:
                bx_sb = sb.tile([128, GC], BF16)
                nc.gpsimd.dma_start(bx_sb[:], bx_d[:])
            if bhh_n_nonzero:
                bhn_sb = sb.tile([128, KH], BF16)
                nc.gpsimd.dma_start(bhn_sb[:], bhn_d[:])
            if bproj_nonzero:
                bpr_sb = sb.tile([128, VS], F32)
                nc.gpsimd.dma_start(bpr_sb[:], bpr_d[:1, :].to_broadcast([128, VS]))

            HT = sb.tile([128, KH, NROW], BF16)     # h_{t+1} states, (t, b) cols
            HT8 = sb.tile([128, KH, NROW], FP8)     # 4x h in fp8 for phase 2

            emit_prep_xg(0, embt0, range(0, GC, 2))

            # ---------------- phase 1 step ----------------------------------
            r_off, z_off, n_off = 0, 4, 8   # gate chunk offsets (r, z, n)

            def emit_step(t):
                h_prev = h0_sb[:, :, :] if t == 0 else HT[:, :, (t - 1) * B:t * B]
                xg = xg_tiles[t // 8][:, t % 8, :, :]
                ps_r = ps.tile([128, 4, B], F32, tag="ps_r", name=f"psr{t}")
                ps_n = ps.tile([128, 4, B], F32, tag="ps_n", name=f"psn{t}")
                ps_z = ps.tile([128, 4, B], F32, tag="ps_z", name=f"psz{t}")

                def gate_mms(dst, off, fold):
                    if fold:
                        nc.tensor.matmul(dst[:], lhsT=ident[:],
                                         rhs=xg[:, off:off + 4, :],
                                         start=True, stop=False)
                    for gc in range(4):
                        for kt in range(KH):
                            nc.tensor.matmul(
                                dst[:, gc, :],
                                lhsT=whh_sb[:, kt, (off + gc) * 128:(off + gc + 1) * 128],
                                rhs=h_prev[:, kt, :],
                                start=(not fold and kt == 0),
                                stop=(kt == KH - 1))

                # burst order: r first (binding n-path needs r_s), then n,
                # then z (its consumers have the most slack).
                gate_mms(ps_r, r_off, True)
                gate_mms(ps_n, n_off, False)
                gate_mms(ps_z, z_off, True)

                # r gate: sigma(x) = 0.5*tanh(x/2) + 0.5  (pre-acts are 16x)
                rt = sb.tile([128, 4, B], BF16, tag="rt", bufs=2, name=f"rt{t}")
                nc.scalar.activation(rt[:], ps_r[:], AF.Tanh,
                                     scale=0.5 / WHH_SCALE)
                r_s = sb.tile([128, 4, B], BF16, tag="r_s", bufs=2, name=f"rs{t}")
                nc.vector.tensor_scalar(out=r_s[:], in0=rt[:], scalar1=0.5,
                                        scalar2=0.5, op0=OP.mult, op1=OP.add)
                # z gate
                zt = sb.tile([128, 4, B], BF16, tag="zt", bufs=2, name=f"zt{t}")
                nc.scalar.activation(zt[:], ps_z[:], AF.Tanh,
                                     scale=0.5 / WHH_SCALE)
                q_s = sb.tile([128, 4, B], BF16, tag="q_s", bufs=2, name=f"qs{t}")
                nc.gpsimd.tensor_scalar(out=q_s[:], in0=zt[:], scalar1=-0.5,
                                        scalar2=0.5, op0=OP.mult, op1=OP.add)
                z_s = sb.tile([128, 4, B], BF16, tag="z_s", bufs=2, name=f"zs{t}")
                nc.gpsimd.tensor_scalar(out=z_s[:], in0=zt[:], scalar1=0.5,
                                        scalar2=0.5, op0=OP.mult, op1=OP.add)
                p_s = sb.tile([128, 4, B], BF16, tag="p_s", bufs=2, name=f"ps{t}")
                nc.gpsimd.tensor_tensor(out=p_s[:], in0=z_s[:], in1=h_prev,
                                        op=OP.mult)
                # n gate: tanh((16*xn + r*(16*hn)) / 16)
                if bhh_n_nonzero:
                    nc.vector.tensor_tensor(
                        out=ps_n[:], in0=ps_n[:],
                        in1=bhn_sb[:, :, None].to_broadcast([128, 4, B]), op=OP.add)
                nc.vector.tensor_tensor(out=ps_n[:], in0=ps_n[:], in1=r_s[:],
                                        op=OP.mult)
                nc.vector.tensor_tensor(out=ps_n[:], in0=ps_n[:],
                                        in1=xg[:, n_off:n_off + 4, :], op=OP.add)
                n_s = sb.tile([128, 4, B], BF16, tag="n_s", bufs=2, name=f"ns{t}")
                nc.scalar.activation(n_s[:], ps_n[:], AF.Tanh,
                                     scale=1.0 / WHH_SCALE)
                # h' = n*(1-z) + z*h
                w_s = sb.tile([128, 4, B], BF16, tag="w_s", bufs=2, name=f"ws{t}")
                nc.vector.tensor_tensor(out=w_s[:], in0=n_s[:], in1=q_s[:],
                                        op=OP.mult)
                nc.vector.tensor_tensor(out=HT[:, :, t * B:(t + 1) * B],
                                        in0=w_s[:], in1=p_s[:], op=OP.add)
                # fp8 copy (x4) for phase 2, off the binding path
                eng = nc.gpsimd if ht8_gps else nc.vector
                eng.tensor_scalar(out=HT8[:, :, t * B:(t + 1) * B],
                                  in0=HT[:, :, t * B:(t + 1) * B],
                                  scalar1=HT8_SCALE, scalar2=None,
                                  op0=OP.mult)

            # ---------------- phase 2 emission helpers ----------------------
            logit_tiles = {}
            lse_tiles = {}

            def emit_munit(m, u):
                # one 1000-vocab unit of row-tile m: DoubleRow fp8 matmuls,
                # f16 raw-logit evacuation, exp + row-sum accumulation.
                mp = min(128, NROW - m * 128)
                if u == 0:
                    logit_tiles[m] = sb.tile([128, VS], F16, tag="logit",
                                             bufs=5, name=f"lg{m}")
                lg = logit_tiles[m]
                pl = ps.tile([128, VU], F32, tag="ps_l", bufs=2,
                             name=f"pl{m}_{u}")
                if use_dr:
                    for khi in range(2):
                        for half in range(2):
                            nc.tensor.matmul(
                                pl[:mp, half * 500:(half + 1) * 500],
                                lhsT=HT8[:, 2 * khi:2 * khi + 2,
                                         m * 128:m * 128 + mp],
                                rhs=wpr_sb[:, 2 * khi:2 * khi + 2,
                                           u * VU + half * 500:u * VU + (half + 1) * 500],
                                start=(khi == 0), stop=(khi == 1),
                                perf_mode=DR)
                else:
                    for half in range(2):
                        for kt in range(KH):
                            nc.tensor.matmul(
                                pl[:mp, half * 500:(half + 1) * 500],
                                lhsT=HT8[:, kt, m * 128:m * 128 + mp],
                                rhs=wpr_sb[:, kt,
                                           u * VU + half * 500:u * VU + (half + 1) * 500],
                                start=(kt == 0), stop=(kt == KH - 1))
                if bproj_nonzero:
                    nc.vector.tensor_tensor(
                        out=pl[:mp], in0=pl[:mp],
                        in1=bpr_sb[:mp, u * VU:(u + 1) * VU], op=OP.add)
                nc.vector.tensor_scalar(
                    out=lg[:mp, u * VU:(u + 1) * VU], in0=pl[:mp],
                    scalar1=1.0 / LG_SCALE, scalar2=None, op0=OP.mult)
                esc = sb.tile([128, VU], F16, tag="exps", bufs=2,
                              name=f"esc{m}_{u}")
                nc.scalar.activation(esc[:mp], pl[:mp], AF.Exp,
                                     bias=ebias[:mp, :1], scale=1.0 / LG_SCALE,
                                     accum_out=S_all[:mp, m * NVU + u:m * NVU + u + 1])

            cc_tiles = {}

            def emit_stats_start(g):
                # local row-sums for the group's 2 row-tiles + AllReduce
                sg = sb.tile([128, MPG], F32, tag="sg", bufs=2, name=f"sg{g}")
                for j in range(MPG):
                    m = g * MPG + j
                    nc.vector.reduce_sum(
                        out=sg[:, j:j + 1],
                        in_=S_all[:, m * NVU:(m + 1) * NVU],
                        axis=mybir.AxisListType.X)
                cin = dp.tile([128, MPG], F32, tag=f"cin{g}", name=f"cin{g}")
                nc.gpsimd.dma_start(cin[:], sg[:])
                cout = dp.tile([128, MPG], F32, tag=f"cout{g}",
                               addr_space="Shared", name=f"cout{g}")
                nc.gpsimd.collective_compute(
                    "AllReduce", OP.add,
                    replica_groups=[list(range(NCORES))],
                    ins=[cin.opt()], outs=[cout.opt()])
                cc_tiles[g] = cout

            def emit_stats_consume(g):
                st = sb.tile([128, MPG], F32, tag="st", bufs=2, name=f"st{g}")
                nc.sync.dma_start(st[:], cc_tiles[g][:])
                # neg_lse = -(e - 127 + 4) * ln2 - ln(m),  St = m * 2^(e-127)
                iu = st[:].bitcast(U32)
                eu = sb.tile([128, MPG], U32, tag="eu", bufs=2, name=f"eu{g}")
                nc.vector.tensor_scalar(out=eu[:], in0=iu, scalar1=23,
                                        scalar2=None, op0=OP.logical_shift_right)
                ef = sb.tile([128, MPG], F32, tag="ef", bufs=2, name=f"ef{g}")
                nc.vector.tensor_copy(ef[:], eu[:])
                mu = sb.tile([128, MPG], U32, tag="mu", bufs=2, name=f"mu{g}")
                nc.vector.tensor_scalar(out=mu[:], in0=iu, scalar1=0x007FFFFF,
                                        scalar2=0x3F800000, op0=OP.bitwise_and,
                                        op1=OP.bitwise_or)
                mf = mu[:].bitcast(F32)
                acc = sb.tile([128, MPG], F32, tag="acc", bufs=2, name=f"acc{g}")
                c = _NEGLN_COEF
                nc.vector.tensor_scalar(out=acc[:], in0=mf, scalar1=c[0],
                                        scalar2=c[1], op0=OP.mult, op1=OP.add)
                for k in range(2, 4):
                    nc.vector.tensor_tensor(out=acc[:], in0=acc[:], in1=mf,
                                            op=OP.mult)
                    nc.vector.tensor_scalar(out=acc[:], in0=acc[:], scalar1=c[k],
                                            scalar2=None, op0=OP.add)
                # + (127 - 4 - e) * ln2   (the -4 re-adds the exp bias)
                e2 = sb.tile([128, MPG], F32, tag="e2", bufs=2, name=f"e2{g}")
                nc.vector.tensor_scalar(out=e2[:], in0=ef[:], scalar1=-LN2,
                                        scalar2=(127.0 - 4.0) * LN2,
                                        op0=OP.mult, op1=OP.add)
                nlse = sb.tile([128, MPG], F32, tag="nlse", bufs=2,
                               name=f"nlse{g}")
                nc.vector.tensor_tensor(out=nlse[:], in0=acc[:], in1=e2[:],
                                        op=OP.add)
                lse_tiles[g] = nlse
                if debug and g == 0:
                    nc.sync.dma_start(nlse_d[:], nlse[:])

            def emit_output(m):
                g, j = m // MPG, m % MPG
                mp = min(128, NROW - m * 128)
                nlse = lse_tiles[g]
                lg = logit_tiles.pop(m)
                if debug and m == 0:
                    nc.sync.dma_start(lg_d[:], lg[:])
                ot = sb.tile([128, VS], F16, tag="ot", bufs=2, name=f"ot{m}")
                # split the +(-lse) pass: half on DVE, half on ACT (per-
                # partition bias) so neither engine eats the full 4000 cols.
                hv = VS // 2
                nc.vector.tensor_tensor(
                    out=ot[:mp, :hv], in0=lg[:mp, :hv],
                    in1=nlse[:mp, j:j + 1].to_broadcast([mp, hv]), op=OP.add)
                nc.scalar.activation(ot[:mp, hv:], lg[:mp, hv:], AF.Identity,
                                     bias=nlse[:mp, j:j + 1])
                nc.sync.dma_start(out_d[m * 128:m * 128 + mp, :], ot[:mp])

            # ---------------- main emission loop ----------------------------
            from collections import deque
            work_q = deque()

            def enqueue_mtile(m):
                # deferred consume of the group started ~4 steps ago: its
                # AllReduce has been in flight since then.
                if m >= 2 and m % 2 == 0:
                    g = (m - 2) // 2

                    def fin(g=g):
                        emit_stats_consume(g)
                        for mm in range(g * MPG, (g + 1) * MPG):
                            emit_output(mm)
                    work_q.append(fin)
                for u in range(NVU):
                    work_q.append(lambda m=m, u=u: emit_munit(m, u))
                if m % 2 == 1:
                    work_q.append(lambda g=m // 2: emit_stats_start(g))

            for t in range(S):
                emit_step(t)
                if t % 8 == 1 and t // 8 + 1 <= (S - 1) // 8:
                    c8 = t // 8 + 1
                    embt = emit_prep_gather(c8)
                    for lo in range(0, GC, 4):
                        work_q.append(lambda c8=c8, embt=embt, lo=lo:
                                      emit_prep_xg(c8, embt,
                                                   range(lo, min(lo + 4, GC), 2)))
                if t >= 3 and (t - 3) % 4 == 0:
                    enqueue_mtile((t - 3) // 4)
                for _ in range(min(3, len(work_q))):
                    work_q.popleft()()
            # tail: tile 15, last group stats, remaining consumes/outputs
            enqueue_mtile(15)
            work_q.append(lambda: emit_stats_consume(7))
            work_q.append(lambda: emit_output(14))
            work_q.append(lambda: emit_output(15))
            while work_q:
                work_q.popleft()()
            if debug:
                nc.sync.dma_start(ht_d[:], HT[:])
                nc.sync.dma_start(sall_d[:], S_all[:])

    nc.finalize()
    _BUILD_CACHE[key] = nc
    return nc


def _pack_T(w, ktiles, scale, np_dt):
    """[out, in] f32 -> [128, ktiles, out] (w.T * scale, k-major slabs)."""
    wT = np.ascontiguousarray(w.T) * scale
    return np.ascontiguousarray(
        wT.reshape(ktiles, 128, w.shape[0]).transpose(1, 0, 2)).astype(np_dt)


LAST_PROFILE = None


def kernel(trg, h0, embed_table, W_ih, W_hh, b_ih, b_hh, W_proj, b_proj):
    global LAST_PROFILE
    trg = np.asarray(trg)
    h0 = np.asarray(h0, dtype=np.float32)
    embed_table = np.asarray(embed_table, dtype=np.float32)
    W_ih = np.asarray(W_ih, dtype=np.float32)
    W_hh = np.asarray(W_hh, dtype=np.float32)
    b_ih = np.asarray(b_ih, dtype=np.float32)
    b_hh = np.asarray(b_hh, dtype=np.float32)
    W_proj = np.asarray(W_proj, dtype=np.float32)
    b_proj = np.asarray(b_proj, dtype=np.float32)

    # bx = b_ih + [b_hh for r,z chunks; 0 for n chunks], scaled like xg
    bx = b_ih.copy()
    bx[:2 * H] += b_hh[:2 * H]
    bx_nonzero = bool(np.any(bx))
    bhh_n_nonzero = bool(np.any(b_hh[2 * H:]))
    bproj_nonzero = bool(np.any(b_proj))
    nc = _build(bx_nonzero, bhh_n_nonzero, bproj_nonzero)

    trg_flat = np.ascontiguousarray(
        trg[:, :S].T.reshape(NROW, 1)).astype(np.int32)
    tbl_bf = embed_table.astype(ml_dtypes.bfloat16)
    wih_t = _pack_T(W_ih, KE, WHH_SCALE, ml_dtypes.bfloat16)
    whh_t = _pack_T(W_hh, KH, WHH_SCALE, NP_FP8)
    h0_t = np.ascontiguousarray(
        h0[0].T.reshape(KH, 128, B).transpose(1, 0, 2)).astype(ml_dtypes.bfloat16)

    base = {
        "trg_flat": trg_flat,
        "emb_tbl": tbl_bf,
        "wih_t": wih_t,
        "whh_t": whh_t,
        "h0_t": h0_t,
    }
    if bx_nonzero:
        base["bx_t"] = np.ascontiguousarray(
            (bx * WHH_SCALE).reshape(GC, 128).T).astype(ml_dtypes.bfloat16)
    if bhh_n_nonzero:
        base["bhn_t"] = np.ascontiguousarray(
            (b_hh[2 * H:] * WHH_SCALE).reshape(KH, 128).T).astype(ml_dtypes.bfloat16)

    in_maps = []
    for c in range(NCORES):
        m = dict(base)
        m["wproj_t"] = _pack_T(W_proj[c * VS:(c + 1) * VS], KH, WPR_SCALE,
                               ml_dtypes.bfloat16)
        if bproj_nonzero:
            m["bproj_s"] = np.ascontiguousarray(
                (b_proj[c * VS:(c + 1) * VS] * LG_SCALE).reshape(1, VS))
        in_maps.append(m)

    trace = bool(int(os.environ.get("KERNEL_TRACE", "0")))
    res = run_bass_kernel_spmd(nc, in_maps, core_ids=list(range(NCORES)),
                               trace=trace)
    LAST_PROFILE = res

    out = np.zeros((B, T, V), dtype=np.float32)
    big = np.stack([res.results[c]["out_lp"].astype(np.float32).reshape(S, B, VS)
                    for c in range(NCORES)], axis=0)   # [c, t, b, vs]
    out[:, 1:, :] = big.transpose(2, 1, 0, 3).reshape(B, S, V)
    return out
